# revision 6
# baseline (speedup 1.0000x reference)
"""Conv2D 3x3 (B=32, C=128, H=W=56 -> OC=256) as a Bass/Tile kernel on 8 NeuronCores.

Strategy: data-parallel over batch (4 images per core), W/b replicated,
1-D Winograd F(2,3) along H. Output row-pairs (2i, 2i+1) are produced from
4 transformed input taps:
  V0 = d0-d2, V1 = d1+d2, V2n = d1-d2 (= -V2), V3 = d1-d3
  (d_k = padded row 2i+k)
  M_t = sum_kw U[t,kw] @ V_t[:, :, kw:kw+56]     (PSUM, 12 matmuls/block
                                                  instead of direct conv's 18)
  y0 = M0 + M1 + M2 + b = m0 + (M1+b) - M2n
  y1 = M1 - M2 - M3 + b = (M1+b) + M2n - M3
U taps (host, from W):  U0=g0, U1=(g0+g1+g2)/2, U2=(g0-g1+g2)/2, U3=g2.

Per (img, 7-row-pair block, oc-half): 12 fp16 matmuls (N=392) accumulate
M0,M1,M2n,M3 into one 4-bank PSUM tile; ACT evacuates them to SBUF with the
bias folded into M1; DVE does 2 paired tensor_tensor passes
  [s0,s1] = [m0,m1b] + [m1b,m2n]   ;   [y0,y1] = [s0,s1] - [m2n,m3]
writing the final row-interleaved fp32 output tile, which DMAs out
contiguously. Input transform: DVE for images 0-1, GpSimd for images 2-3
(keeps DVE under the PE time). PE work drops from ~97us (direct) to ~65us.
"""

import os

import numpy as np

import concourse.bacc as bacc
import concourse.mybir as mybir
import concourse.tile as tile
from concourse import bass_utils

B, C, H, W_SP = 32, 128, 56, 56
OC, KH, KW = 256, 3, 3
N_CORES = 8
B_PER = B // N_CORES            # 4 images per core
HP, WP = H + 2, W_SP + 2        # zero-padded spatial dims (58x58)
HWO = H * W_SP                  # 3136
NP = H // 2                     # 28 output row-pairs
PB = 7                          # row-pairs per block
NBLK = NP // PB                 # 4 blocks per image
NT = PB * W_SP                  # 392 free columns per matmul
OC_TILES = OC // 128            # 2
NTAP = 4
# V tile tap order is (V0, V3, V1, V2n) so the paired DVE/GPS input-transform
# op can write V0/V3 contiguously; PSUM tap t -> V slot:
VMAP = [0, 2, 3, 1]             # t0->V0, t1->V1(slot2), t2->V2n(slot3), t3->V3(slot1)

_NC_CACHE: dict[str, object] = {}


def _mm_mode() -> str:
    return os.environ.get("CONV_MM_DTYPE", "f16")


def _build_nc(mode: str):
    in_dt = {
        "bf16": mybir.dt.bfloat16,
        "f16": mybir.dt.float16,
        "f32r": mybir.dt.float32r,
        "f32": mybir.dt.float32,
    }[mode]
    nc = bacc.Bacc(
        "TRN2",
        target_bir_lowering=False,
        debug=False,
        enable_asserts=False,
        num_devices=N_CORES,
    )
    xp = nc.dram_tensor("xp", [B_PER, C, HP * WP], in_dt, kind="ExternalInput").ap()
    wt = nc.dram_tensor(
        "wt", [C, NTAP * KW * OC], in_dt, kind="ExternalInput"
    ).ap()
    bias = nc.dram_tensor(
        "bias", [128, OC_TILES], mybir.dt.float32, kind="ExternalInput"
    ).ap()
    out = nc.dram_tensor(
        "out", [B_PER, OC, HWO], mybir.dt.float32, kind="ExternalOutput"
    ).ap()

    ident = mybir.ActivationFunctionType.Identity
    copyf = mybir.ActivationFunctionType.Copy

    with tile.TileContext(nc) as tc:
        with (
            tc.tile_pool(name="xin", bufs=4) as xpool,
            tc.tile_pool(name="vpool", bufs=4) as vpool,
            tc.tile_pool(name="wpool", bufs=1) as wpool,
            tc.tile_pool(name="bpool", bufs=1) as bpool,
            tc.tile_pool(name="mpool", bufs=3) as mpool,
            tc.tile_pool(name="spool", bufs=3) as spool,
            tc.tile_pool(name="opool", bufs=6) as opool,
            tc.tile_pool(name="psum", bufs=2, space="PSUM") as pspool,
        ):
            # HAM warm-up: burn matmuls on a zeroed tile while the lead-in
            # DMAs are in flight so the real MM stream starts at 2.4 GHz.
            # memset on DVE (its preamble finishes early; gpsimd's is late).
            wu = wpool.tile([C, 512], in_dt, tag="wu")
            nc.vector.memset(wu[:], 0.0)
            psw = pspool.tile([128, NTAP, 512], mybir.dt.float32, tag="ps")
            for i in range(8):
                nc.tensor.matmul(
                    psw[:, 0, :],
                    wu[:, :128],
                    wu[:],
                    start=(i == 0),
                    stop=(i == 7),
                )

            # lead-in DMAs, finest first: weight tap group 0, first half of
            # image 0, then the rest. Bias rides GpSimd.
            wsb = wpool.tile([C, NTAP * KW, OC], in_dt, tag="wsb")
            wtv = wt.rearrange("c (k m) -> c k m", m=OC)
            nc.sync.dma_start(wsb[:, 0:KW, :], wtv[:, 0:KW, :])

            xts = []
            for img in range(B_PER):
                xts.append(
                    xpool.tile([C, HP, WP], in_dt, tag="xc", name=f"xc{img}")
                )
            xviews = [xp[img].rearrange("c (h w) -> c h w", w=WP) for img in range(B_PER)]
            # image 0 in two halves (shorter critical path to first matmul)
            nc.sync.dma_start(xts[0][:, 0:30, :], xviews[0][:, 0:30, :])
            nc.sync.dma_start(wsb[:, KW : NTAP * KW, :], wtv[:, KW : NTAP * KW, :])
            nc.sync.dma_start(xts[0][:, 30:HP, :], xviews[0][:, 30:HP, :])
            nc.sync.dma_start(xts[1][:, 0:30, :], xviews[1][:, 0:30, :])
            nc.sync.dma_start(xts[1][:, 30:HP, :], xviews[1][:, 30:HP, :])
            nc.sync.dma_start(xts[2][:], xviews[2][:])
            nc.sync.dma_start(xts[3][:], xviews[3][:])

            bsb = bpool.tile([128, OC_TILES], mybir.dt.float32, tag="bsb")
            nc.gpsimd.dma_start(bsb[:], bias[:])

            # ---- input transform ----
            # V[c, slot, i, w], slots (V0, V3, V1, V2n).
            # xv2[c, a, i2, w] = x[c, 2*i2 + a, w]
            vts = []
            for img in range(B_PER):
                vts.append(
                    vpool.tile(
                        [C, NTAP, NP, WP], in_dt, tag="vt", name=f"vt{img}"
                    )
                )

            def v_ops(img, p0, np_, eng):
                xv2 = xts[img].rearrange("c (i2 a) w -> c a i2 w", a=2)
                vt = vts[img]
                # paired: V0 = d0 - d2 (a=0), V3 = d1 - d3 (a=1)
                eng.tensor_sub(
                    vt[:, 0:2, p0 : p0 + np_, :],
                    xv2[:, :, p0 : p0 + np_, :],
                    xv2[:, :, p0 + 1 : p0 + np_ + 1, :],
                )
                # V1 = d1 + d2
                eng.tensor_add(
                    vt[:, 2, p0 : p0 + np_, :],
                    xv2[:, 1, p0 : p0 + np_, :],
                    xv2[:, 0, p0 + 1 : p0 + np_ + 1, :],
                )
                # V2n = d1 - d2
                eng.tensor_sub(
                    vt[:, 3, p0 : p0 + np_, :],
                    xv2[:, 1, p0 : p0 + np_, :],
                    xv2[:, 0, p0 + 1 : p0 + np_ + 1, :],
                )

            # image 0 on DVE in two chunks (follows the two x half-DMAs)
            v_ops(0, 0, 14, nc.vector)
            v_ops(0, 14, 14, nc.vector)
            # images 2,3 on GpSimd (otherwise idle; frees DVE for the combine)
            v_ops(2, 0, NP, nc.gpsimd)
            v_ops(3, 0, NP, nc.gpsimd)

            # ---- main loop ----
            def process(img, i0, np_, oc_t, vt):
                n = np_ * W_SP
                ps = pspool.tile([128, NTAP, 512], mybir.dt.float32, tag="ps")
                for t in range(NTAP):
                    vs = VMAP[t]
                    for kw in range(KW):
                        nc.tensor.matmul(
                            ps[:, t, :n],
                            wsb[:, t * KW + kw, oc_t * 128 : (oc_t + 1) * 128],
                            vt[:, vs, i0 : i0 + np_, kw : kw + W_SP],
                            start=(kw == 0),
                            stop=(kw == KW - 1),
                        )
                msb = mpool.tile([128, NTAP, NT], mybir.dt.float32, tag="m")
                # evacuate taps; bias rides the M1 pass
                nc.scalar.activation(msb[:, 2:4, :n], ps[:, 2:4, :n], copyf)
                nc.scalar.activation(msb[:, 0, :n], ps[:, 0, :n], copyf)
                nc.scalar.activation(
                    msb[:, 1, :n],
                    ps[:, 1, :n],
                    ident,
                    bias=bsb[:, oc_t : oc_t + 1],
                )
                st = spool.tile([128, 2, NT], mybir.dt.float32, tag="s")
                nc.vector.tensor_add(
                    st[:, :, :n], msb[:, 0:2, :n], msb[:, 1:3, :n]
                )
                ot = opool.tile([128, 2 * NT], mybir.dt.float32, tag="ot")
                otv = ot.rearrange("p (i j w) -> p j i w", j=2, w=W_SP)
                nc.vector.tensor_sub(
                    otv[:, :, :np_, :], st[:, :, :n], msb[:, 2:4, :n]
                )
                col0 = i0 * 2 * W_SP
                nc.sync.dma_start(
                    out[
                        img,
                        oc_t * 128 : (oc_t + 1) * 128,
                        col0 : col0 + 2 * n,
                    ],
                    ot[:, : 2 * n],
                )

            for img in range(B_PER):
                for blk in range(NBLK):
                    for oc_t in range(OC_TILES):
                        last = (
                            img == B_PER - 1
                            and blk == NBLK - 1
                            and oc_t == OC_TILES - 1
                        )
                        i0 = blk * PB
                        if last:
                            # split the final block so its first half's
                            # ACT/DVE/DMA chain overlaps the second half's
                            # matmuls (shortens the end-of-kernel drain)
                            process(img, i0, 4, oc_t, vts[img])
                            process(img, i0 + 4, 3, oc_t, vts[img])
                        else:
                            process(img, i0, PB, oc_t, vts[img])
                    if img == 0 and blk == 0:
                        # image 1's input transform on DVE, emitted here so
                        # the DVE queue runs img0-block0's combine first
                        v_ops(1, 0, 14, nc.vector)
                        v_ops(1, 14, 14, nc.vector)
    nc.compile()
    return nc


def _get_nc(mode: str):
    nc = _NC_CACHE.get(mode)
    if nc is None:
        nc = _build_nc(mode)
        _NC_CACHE[mode] = nc
    return nc


def kernel(x: np.ndarray, W: np.ndarray, b: np.ndarray) -> np.ndarray:
    mode = _mm_mode()
    x = np.asarray(x, dtype=np.float32)
    W = np.asarray(W, dtype=np.float32)
    b = np.asarray(b, dtype=np.float32)

    if mode == "bf16":
        import ml_dtypes

        in_np_dt = ml_dtypes.bfloat16
    elif mode == "f16":
        in_np_dt = np.float16
    else:
        in_np_dt = np.float32

    # Host-side layout prep: zero-pad x spatially; build the Winograd weight
    # taps U[t,kw] in lhsT layout [c, (t*KW+kw)*OC + oc]; stripe bias.
    xp = np.zeros((B, C, HP, WP), dtype=in_np_dt)
    xp[:, :, 1:-1, 1:-1] = x
    xp = xp.reshape(N_CORES, B_PER, C, HP * WP)

    g = W.reshape(OC, C, KH, KW)
    g0, g1, g2 = g[:, :, 0, :], g[:, :, 1, :], g[:, :, 2, :]
    u = np.stack(
        [g0, (g0 + g1 + g2) * 0.5, (g0 - g1 + g2) * 0.5, g2], axis=0
    )  # [t, OC, C, KW]
    wt = np.ascontiguousarray(
        u.transpose(2, 0, 3, 1).reshape(C, NTAP * KW * OC)
    ).astype(in_np_dt)
    bias = np.ascontiguousarray(b.reshape(OC_TILES, 128).T).astype(np.float32)

    nc = _get_nc(mode)
    in_maps = [
        {"xp": np.ascontiguousarray(xp[i]), "wt": wt, "bias": bias}
        for i in range(N_CORES)
    ]
    trace = os.environ.get("CONV_TRACE", "") not in ("", "0")
    try:
        res = bass_utils.run_bass_kernel_spmd(
            nc,
            in_maps,
            core_ids=list(range(N_CORES)),
            trace=trace,
        )
    except Exception:
        # transient device wedges (NRT_EXEC_UNIT_UNRECOVERABLE) have been
        # observed once; a fresh dispatch usually recovers
        import time

        time.sleep(2.0)
        res = bass_utils.run_bass_kernel_spmd(
            nc,
            in_maps,
            core_ids=list(range(N_CORES)),
            trace=trace,
        )
    kernel._last_results = res  # for test harness introspection
    out = np.stack([res.results[i]["out"] for i in range(N_CORES)])
    return out.reshape(B, OC, H, W_SP)


# revision 13
# speedup vs baseline: 1.1419x; 1.1419x over previous
"""Conv2D 3x3 (B=32, C=128, H=W=56 -> OC=256) as a Bass/Tile kernel on 8 NeuronCores.

Strategy: data-parallel over batch (4 images per core), W/b replicated,
1-D Winograd F(2,3) along H. Output row-pairs (2i, 2i+1) are produced from
4 transformed input taps:
  V0 = d0-d2, V1 = d1+d2, V2n = d1-d2 (= -V2), V3 = d1-d3
  (d_k = padded row 2i+k)
  M_t = sum_kw U[t,kw] @ V_t[:, :, kw:kw+56]     (PSUM, 12 matmuls/block
                                                  instead of direct conv's 18)
  y0 = M0 + M1 + M2 + b = m0 + (M1+b) - M2n
  y1 = M1 - M2 - M3 + b = (M1+b) + M2n - M3
U taps (host, from W):  U0=g0, U1=(g0+g1+g2)/2, U2=(g0-g1+g2)/2, U3=g2.

Per (img, 7-row-pair block, oc-half): 12 fp16 matmuls (N=392) accumulate
M0,M1,M2n,M3 into one 4-bank PSUM tile; ACT evacuates them to SBUF with the
bias folded into M1; DVE does 2 paired tensor_tensor passes
  [s0,s1] = [m0,m1b] + [m1b,m2n]   ;   [y0,y1] = [s0,s1] - [m2n,m3]
writing the final row-interleaved fp32 output tile, which DMAs out
contiguously. Input transform: DVE for images 0-1, GpSimd for images 2-3
(keeps DVE under the PE time). PE work drops from ~97us (direct) to ~65us.
"""

import os

import numpy as np

import concourse.bacc as bacc
import concourse.mybir as mybir
import concourse.tile as tile
from concourse import bass_utils

B, C, H, W_SP = 32, 128, 56, 56
OC, KH, KW = 256, 3, 3
N_CORES = 8
B_PER = B // N_CORES            # 4 images per core
HP, WP = H + 2, W_SP + 2        # zero-padded spatial dims (58x58)
HWO = H * W_SP                  # 3136
NP = H // 2                     # 28 output row-pairs
PB = 7                          # row-pairs per block
NBLK = NP // PB                 # 4 blocks per image
NT = PB * W_SP                  # 392 free columns per matmul
OC_TILES = OC // 128            # 2
NTAP = 4
# V tile tap order is (V0, V3, V1, V2n) so the paired DVE/GPS input-transform
# op can write V0/V3 contiguously; PSUM tap t -> V slot:
VMAP = [0, 2, 3, 1]             # t0->V0, t1->V1(slot2), t2->V2n(slot3), t3->V3(slot1)

_NC_CACHE: dict[str, object] = {}


def _mm_mode() -> str:
    return os.environ.get("CONV_MM_DTYPE", "f16")


def _build_nc(mode: str):
    in_dt = {
        "bf16": mybir.dt.bfloat16,
        "f16": mybir.dt.float16,
        "f32r": mybir.dt.float32r,
        "f32": mybir.dt.float32,
    }[mode]
    nc = bacc.Bacc(
        "TRN2",
        target_bir_lowering=False,
        debug=False,
        enable_asserts=False,
        num_devices=N_CORES,
    )
    xp = nc.dram_tensor("xp", [B_PER, C, HP * WP], in_dt, kind="ExternalInput").ap()
    wt = nc.dram_tensor(
        "wt", [C, NTAP * KW * OC], in_dt, kind="ExternalInput"
    ).ap()
    bias = nc.dram_tensor(
        "bias", [128, OC_TILES], mybir.dt.float32, kind="ExternalInput"
    ).ap()
    out = nc.dram_tensor(
        "out", [B_PER, OC, HWO], mybir.dt.float32, kind="ExternalOutput"
    ).ap()

    ident = mybir.ActivationFunctionType.Identity
    copyf = mybir.ActivationFunctionType.Copy

    with tile.TileContext(nc) as tc:
        with (
            tc.tile_pool(name="xin", bufs=4) as xpool,
            tc.tile_pool(name="vpool", bufs=4) as vpool,
            tc.tile_pool(name="wpool", bufs=1) as wpool,
            tc.tile_pool(name="bpool", bufs=1) as bpool,
            tc.tile_pool(name="mpool", bufs=4) as mpool,
            tc.tile_pool(name="spool", bufs=3) as spool,
            tc.tile_pool(name="opool", bufs=6) as opool,
            tc.tile_pool(name="psum", bufs=4, space="PSUM") as pspool,
        ):
            # HAM warm-up: burn matmuls on a zeroed tile while the lead-in
            # DMAs are in flight so the real MM stream starts at 2.4 GHz.
            # memset on DVE (its preamble finishes early; gpsimd's is late).
            wu = wpool.tile([C, 512], in_dt, tag="wu")
            nc.vector.memset(wu[:], 0.0)
            psw = pspool.tile([128, 2, 512], mybir.dt.float32, tag="ps")
            for i in range(8):
                nc.tensor.matmul(
                    psw[:, 0, :],
                    wu[:, :128],
                    wu[:],
                    start=(i == 0),
                    stop=(i == 7),
                )

            # lead-in DMAs, finest first: weight tap group 0, first half of
            # image 0, then the rest. Bias rides GpSimd.
            wsb = wpool.tile([C, NTAP * KW, OC], in_dt, tag="wsb")
            wtv = wt.rearrange("c (k m) -> c k m", m=OC)
            nc.sync.dma_start(wsb[:, 0:KW, :], wtv[:, 0:KW, :])

            xts = []
            for img in range(B_PER):
                xts.append(
                    xpool.tile([C, HP, WP], in_dt, tag="xc", name=f"xc{img}")
                )
            xviews = [xp[img].rearrange("c (h w) -> c h w", w=WP) for img in range(B_PER)]
            # image 0 in two halves (shorter critical path to first matmul)
            nc.sync.dma_start(xts[0][:, 0:30, :], xviews[0][:, 0:30, :])
            nc.sync.dma_start(wsb[:, KW : NTAP * KW, :], wtv[:, KW : NTAP * KW, :])
            nc.sync.dma_start(xts[0][:, 30:HP, :], xviews[0][:, 30:HP, :])
            nc.sync.dma_start(xts[1][:, 0:30, :], xviews[1][:, 0:30, :])
            nc.sync.dma_start(xts[1][:, 30:HP, :], xviews[1][:, 30:HP, :])
            nc.sync.dma_start(xts[2][:], xviews[2][:])
            nc.sync.dma_start(xts[3][:], xviews[3][:])

            bsb = bpool.tile([128, OC_TILES], mybir.dt.float32, tag="bsb")
            nc.gpsimd.dma_start(bsb[:], bias[:])

            # ---- input transform ----
            # V[c, slot, i, w], slots (V0, V3, V1, V2n).
            # xv2[c, a, i2, w] = x[c, 2*i2 + a, w]
            vts = []
            for img in range(B_PER):
                vts.append(
                    vpool.tile(
                        [C, NTAP, NP, WP], in_dt, tag="vt", name=f"vt{img}"
                    )
                )

            def v_ops(img, p0, np_, eng):
                # single-tap 2-free-dim ops: 3+ free dims fall off the DVE
                # 2x fast path (measured ~4x slower), singles hit it
                xv2 = xts[img].rearrange("c (i2 a) w -> c a i2 w", a=2)
                vt = vts[img]
                lo = slice(p0, p0 + np_)
                hi = slice(p0 + 1, p0 + np_ + 1)
                # V0 = d0 - d2
                eng.tensor_sub(vt[:, 0, lo, :], xv2[:, 0, lo, :], xv2[:, 0, hi, :])
                # V3 = d1 - d3
                eng.tensor_sub(vt[:, 1, lo, :], xv2[:, 1, lo, :], xv2[:, 1, hi, :])
                # V1 = d1 + d2
                eng.tensor_add(vt[:, 2, lo, :], xv2[:, 1, lo, :], xv2[:, 0, hi, :])
                # V2n = d1 - d2
                eng.tensor_sub(vt[:, 3, lo, :], xv2[:, 1, lo, :], xv2[:, 0, hi, :])

            # images 0,1 on DVE, chunked to follow the half-image DMAs
            v_ops(0, 0, 14, nc.vector)
            v_ops(0, 14, 14, nc.vector)
            v_ops(1, 0, 14, nc.vector)
            v_ops(1, 14, 14, nc.vector)
            # images 2,3 on GpSimd (otherwise idle; frees DVE for the combine)
            v_ops(2, 0, NP, nc.gpsimd)
            v_ops(3, 0, NP, nc.gpsimd)

            # ---- main loop ----
            def process(img, i0, np_, oc_t, vt):
                n = np_ * W_SP
                # two 2-bank PSUM tiles per block: taps 0,1 in psa, 2,3 in
                # psb. Their single-tap evacs run as soon as each group
                # finishes, releasing banks early for the PE.
                psa = pspool.tile([128, 2, 512], mybir.dt.float32, tag="ps", name="psa")
                psb = pspool.tile([128, 2, 512], mybir.dt.float32, tag="ps", name="psb")
                for t in range(NTAP):
                    vs = VMAP[t]
                    ps = psa if t < 2 else psb
                    for kw in range(KW):
                        nc.tensor.matmul(
                            ps[:, t % 2, :n],
                            wsb[:, t * KW + kw, oc_t * 128 : (oc_t + 1) * 128],
                            vt[:, vs, i0 : i0 + np_, kw : kw + W_SP],
                            start=(kw == 0),
                            stop=(kw == KW - 1),
                        )
                msb = mpool.tile([128, NTAP, NT], mybir.dt.float32, tag="m")
                # evacuate taps in group-completion order; bias rides M1
                nc.scalar.activation(msb[:, 0, :n], psa[:, 0, :n], copyf)
                nc.scalar.activation(
                    msb[:, 1, :n],
                    psa[:, 1, :n],
                    ident,
                    bias=bsb[:, oc_t : oc_t + 1],
                )
                nc.scalar.activation(msb[:, 2:4, :n], psb[:, 0:2, :n], copyf)
                st = spool.tile([128, 2, NT], mybir.dt.float32, tag="s")
                nc.vector.tensor_add(
                    st[:, :, :n], msb[:, 0:2, :n], msb[:, 1:3, :n]
                )
                ot = opool.tile([128, 2 * NT], mybir.dt.float32, tag="ot")
                otv = ot.rearrange("p (i j w) -> p j i w", j=2, w=W_SP)
                nc.vector.tensor_sub(
                    otv[:, :, :np_, :], st[:, :, :n], msb[:, 2:4, :n]
                )
                col0 = i0 * 2 * W_SP
                nc.sync.dma_start(
                    out[
                        img,
                        oc_t * 128 : (oc_t + 1) * 128,
                        col0 : col0 + 2 * n,
                    ],
                    ot[:, : 2 * n],
                )

            for img in range(B_PER):
                for blk in range(NBLK):
                    for oc_t in range(OC_TILES):
                        last = (
                            img == B_PER - 1
                            and blk == NBLK - 1
                            and oc_t == OC_TILES - 1
                        )
                        i0 = blk * PB
                        if last:
                            # split the final block so its first half's
                            # ACT/DVE/DMA chain overlaps the second half's
                            # matmuls (shortens the end-of-kernel drain)
                            process(img, i0, 4, oc_t, vts[img])
                            process(img, i0 + 4, 3, oc_t, vts[img])
                        else:
                            process(img, i0, PB, oc_t, vts[img])

    nc.compile()
    return nc


def _get_nc(mode: str):
    nc = _NC_CACHE.get(mode)
    if nc is None:
        nc = _build_nc(mode)
        _NC_CACHE[mode] = nc
    return nc


def kernel(x: np.ndarray, W: np.ndarray, b: np.ndarray) -> np.ndarray:
    mode = _mm_mode()
    x = np.asarray(x, dtype=np.float32)
    W = np.asarray(W, dtype=np.float32)
    b = np.asarray(b, dtype=np.float32)

    if mode == "bf16":
        import ml_dtypes

        in_np_dt = ml_dtypes.bfloat16
    elif mode == "f16":
        in_np_dt = np.float16
    else:
        in_np_dt = np.float32

    # Host-side layout prep: zero-pad x spatially; build the Winograd weight
    # taps U[t,kw] in lhsT layout [c, (t*KW+kw)*OC + oc]; stripe bias.
    xp = np.zeros((B, C, HP, WP), dtype=in_np_dt)
    xp[:, :, 1:-1, 1:-1] = x
    xp = xp.reshape(N_CORES, B_PER, C, HP * WP)

    g = W.reshape(OC, C, KH, KW)
    g0, g1, g2 = g[:, :, 0, :], g[:, :, 1, :], g[:, :, 2, :]
    u = np.stack(
        [g0, (g0 + g1 + g2) * 0.5, (g0 - g1 + g2) * 0.5, g2], axis=0
    )  # [t, OC, C, KW]
    wt = np.ascontiguousarray(
        u.transpose(2, 0, 3, 1).reshape(C, NTAP * KW * OC)
    ).astype(in_np_dt)
    bias = np.ascontiguousarray(b.reshape(OC_TILES, 128).T).astype(np.float32)

    nc = _get_nc(mode)
    in_maps = [
        {"xp": np.ascontiguousarray(xp[i]), "wt": wt, "bias": bias}
        for i in range(N_CORES)
    ]
    trace = os.environ.get("CONV_TRACE", "") not in ("", "0")
    try:
        res = bass_utils.run_bass_kernel_spmd(
            nc,
            in_maps,
            core_ids=list(range(N_CORES)),
            trace=trace,
        )
    except Exception:
        # transient device wedges (NRT_EXEC_UNIT_UNRECOVERABLE) have been
        # observed once; a fresh dispatch usually recovers
        import time

        time.sleep(2.0)
        res = bass_utils.run_bass_kernel_spmd(
            nc,
            in_maps,
            core_ids=list(range(N_CORES)),
            trace=trace,
        )
    kernel._last_results = res  # for test harness introspection
    out = np.stack([res.results[i]["out"] for i in range(N_CORES)])
    return out.reshape(B, OC, H, W_SP)


# revision 16
# speedup vs baseline: 1.3353x; 1.1693x over previous
"""Conv2D 3x3 (B=32, C=128, H=W=56 -> OC=256) as a Bass/Tile kernel on 8 NeuronCores.

Strategy: data-parallel over batch (4 images per core), W/b replicated,
1-D Winograd F(2,3) along H. Output row-pairs (2i, 2i+1) are produced from
4 transformed input taps:
  V0 = d0-d2, V1 = d1+d2, V2n = d1-d2 (= -V2), V3 = d1-d3
  (d_k = padded row 2i+k)
  M_t = sum_kw U[t,kw] @ V_t[:, :, kw:kw+56]     (PSUM, 12 matmuls/block
                                                  instead of direct conv's 18)
  y0 = M0 + M1 + M2 + b = m0 + (M1+b) - M2n
  y1 = M1 - M2 - M3 + b = (M1+b) + M2n - M3
U taps (host, from W):  U0=g0, U1=(g0+g1+g2)/2, U2=(g0-g1+g2)/2, U3=g2.

Per (img, 7-row-pair block, oc-half): 12 fp16 matmuls (N=392) accumulate
M0,M1,M2n,M3 into one 4-bank PSUM tile; ACT evacuates them to SBUF with the
bias folded into M1; DVE does 2 paired tensor_tensor passes
  [s0,s1] = [m0,m1b] + [m1b,m2n]   ;   [y0,y1] = [s0,s1] - [m2n,m3]
writing the final row-interleaved fp32 output tile, which DMAs out
contiguously. Input transform: DVE for images 0-1, GpSimd for images 2-3
(keeps DVE under the PE time). PE work drops from ~97us (direct) to ~65us.
"""

import os

import numpy as np

import concourse.bacc as bacc
import concourse.mybir as mybir
import concourse.tile as tile
from concourse import bass_utils

B, C, H, W_SP = 32, 128, 56, 56
OC, KH, KW = 256, 3, 3
N_CORES = 8
B_PER = B // N_CORES            # 4 images per core
HP, WP = H + 2, W_SP + 2        # zero-padded spatial dims (58x58)
HWO = H * W_SP                  # 3136
NP = H // 2                     # 28 output row-pairs
PB = 7                          # row-pairs per block
NBLK = NP // PB                 # 4 blocks per image
NT = PB * W_SP                  # 392 free columns per matmul
OC_TILES = OC // 128            # 2
NTAP = 4
# V tile tap order is (V0, V3, V1, V2n) so the paired DVE/GPS input-transform
# op can write V0/V3 contiguously; PSUM tap t -> V slot:
VMAP = [0, 2, 3, 1]             # t0->V0, t1->V1(slot2), t2->V2n(slot3), t3->V3(slot1)

_NC_CACHE: dict[str, object] = {}


def _mm_mode() -> str:
    return os.environ.get("CONV_MM_DTYPE", "f16")


def _build_nc(mode: str):
    in_dt = {
        "bf16": mybir.dt.bfloat16,
        "f16": mybir.dt.float16,
        "f32r": mybir.dt.float32r,
        "f32": mybir.dt.float32,
    }[mode]
    nc = bacc.Bacc(
        "TRN2",
        target_bir_lowering=False,
        debug=False,
        enable_asserts=False,
        num_devices=N_CORES,
    )
    xp = nc.dram_tensor("xp", [B_PER, C, HP * WP], in_dt, kind="ExternalInput").ap()
    wt = nc.dram_tensor(
        "wt", [C, NTAP * KW * OC], in_dt, kind="ExternalInput"
    ).ap()
    bias = nc.dram_tensor(
        "bias", [128, OC_TILES], mybir.dt.float32, kind="ExternalInput"
    ).ap()
    out = nc.dram_tensor(
        "out", [B_PER, OC, HWO], mybir.dt.float32, kind="ExternalOutput"
    ).ap()

    ident = mybir.ActivationFunctionType.Identity
    copyf = mybir.ActivationFunctionType.Copy

    with tile.TileContext(nc) as tc:
        with (
            tc.tile_pool(name="xin", bufs=4) as xpool,
            tc.tile_pool(name="vpool", bufs=4) as vpool,
            tc.tile_pool(name="wpool", bufs=1) as wpool,
            tc.tile_pool(name="bpool", bufs=1) as bpool,
            tc.tile_pool(name="mpool", bufs=5) as mpool,
            tc.tile_pool(name="spool", bufs=3) as spool,
            tc.tile_pool(name="opool", bufs=6) as opool,
            tc.tile_pool(name="psum", bufs=4, space="PSUM") as pspool,
        ):
            # HAM warm-up: burn matmuls on a zeroed tile while the lead-in
            # DMAs are in flight so the real MM stream starts at 2.4 GHz.
            # memset on DVE (its preamble finishes early; gpsimd's is late).
            wu = wpool.tile([C, 512], in_dt, tag="wu")
            nc.vector.memset(wu[:], 0.0)
            psw = pspool.tile([128, 2, 512], mybir.dt.float32, tag="ps")
            for i in range(8):
                nc.tensor.matmul(
                    psw[:, 0, :],
                    wu[:, :128],
                    wu[:],
                    start=(i == 0),
                    stop=(i == 7),
                )

            # lead-in DMAs, finest first: weight tap group 0, first half of
            # image 0, then the rest. Bias rides GpSimd.
            wsb = wpool.tile([C, NTAP * KW, OC], in_dt, tag="wsb")
            wtv = wt.rearrange("c (k m) -> c k m", m=OC)
            nc.sync.dma_start(wsb[:, 0:KW, :], wtv[:, 0:KW, :])

            xts = []
            for img in range(B_PER):
                xts.append(
                    xpool.tile([C, HP, WP], in_dt, tag="xc", name=f"xc{img}")
                )
            xviews = [xp[img].rearrange("c (h w) -> c h w", w=WP) for img in range(B_PER)]
            # image 0 in two halves (shorter critical path to first matmul)
            nc.sync.dma_start(xts[0][:, 0:30, :], xviews[0][:, 0:30, :])
            nc.sync.dma_start(wsb[:, KW : NTAP * KW, :], wtv[:, KW : NTAP * KW, :])
            nc.sync.dma_start(xts[0][:, 30:HP, :], xviews[0][:, 30:HP, :])
            nc.sync.dma_start(xts[1][:, 0:30, :], xviews[1][:, 0:30, :])
            nc.sync.dma_start(xts[1][:, 30:HP, :], xviews[1][:, 30:HP, :])
            nc.sync.dma_start(xts[2][:], xviews[2][:])
            nc.sync.dma_start(xts[3][:], xviews[3][:])

            bsb = bpool.tile([128, OC_TILES], mybir.dt.float32, tag="bsb")
            nc.gpsimd.dma_start(bsb[:], bias[:])

            # ---- input transform ----
            # V[c, slot, i, w], slots (V0, V3, V1, V2n).
            # xv2[c, a, i2, w] = x[c, 2*i2 + a, w]
            vts = []
            for img in range(B_PER):
                vts.append(
                    vpool.tile(
                        [C, NTAP, NP, WP], in_dt, tag="vt", name=f"vt{img}"
                    )
                )

            def v_ops(img, p0, np_, eng):
                # single-tap 2-free-dim ops: 3+ free dims fall off the DVE
                # 2x fast path (measured ~4x slower), singles hit it
                xv2 = xts[img].rearrange("c (i2 a) w -> c a i2 w", a=2)
                vt = vts[img]
                lo = slice(p0, p0 + np_)
                hi = slice(p0 + 1, p0 + np_ + 1)
                # V0 = d0 - d2
                eng.tensor_sub(vt[:, 0, lo, :], xv2[:, 0, lo, :], xv2[:, 0, hi, :])
                # V3 = d1 - d3
                eng.tensor_sub(vt[:, 1, lo, :], xv2[:, 1, lo, :], xv2[:, 1, hi, :])
                # V1 = d1 + d2
                eng.tensor_add(vt[:, 2, lo, :], xv2[:, 1, lo, :], xv2[:, 0, hi, :])
                # V2n = d1 - d2
                eng.tensor_sub(vt[:, 3, lo, :], xv2[:, 1, lo, :], xv2[:, 0, hi, :])

            # All input transforms on DVE: GpSimd tensor ops are ~4x slower,
            # pay a ~20us first-op IRAM load, and contend with DVE for the
            # shared SBUF port (measured quartering DVE throughput).
            # Images 0,1 chunked to follow the half-image DMAs; images 2,3
            # are emitted inside the block loop so the DVE FIFO reaches them
            # roughly when their x DMAs land.
            v_ops(0, 0, 14, nc.vector)
            v_ops(0, 14, 14, nc.vector)
            v_ops(1, 0, 14, nc.vector)
            v_ops(1, 14, 14, nc.vector)

            # ---- main loop ----
            def process(img, i0, np_, oc_t, vt):
                n = np_ * W_SP
                # two 2-bank PSUM tiles per block: taps 0,1 in psa, 2,3 in
                # psb. Their single-tap evacs run as soon as each group
                # finishes, releasing banks early for the PE.
                psa = pspool.tile([128, 2, 512], mybir.dt.float32, tag="ps", name="psa")
                psb = pspool.tile([128, 2, 512], mybir.dt.float32, tag="ps", name="psb")
                for t in range(NTAP):
                    vs = VMAP[t]
                    ps = psa if t < 2 else psb
                    for kw in range(KW):
                        nc.tensor.matmul(
                            ps[:, t % 2, :n],
                            wsb[:, t * KW + kw, oc_t * 128 : (oc_t + 1) * 128],
                            vt[:, vs, i0 : i0 + np_, kw : kw + W_SP],
                            start=(kw == 0),
                            stop=(kw == KW - 1),
                        )
                msb = mpool.tile([128, NTAP, NT], mybir.dt.float32, tag="m")
                # evacuate taps in group-completion order; bias rides M1
                nc.scalar.activation(msb[:, 0, :n], psa[:, 0, :n], copyf)
                nc.scalar.activation(
                    msb[:, 1, :n],
                    psa[:, 1, :n],
                    ident,
                    bias=bsb[:, oc_t : oc_t + 1],
                )
                nc.scalar.activation(msb[:, 2:4, :n], psb[:, 0:2, :n], copyf)
                st = spool.tile([128, 2, NT], mybir.dt.float32, tag="s")
                nc.vector.tensor_add(
                    st[:, :, :n], msb[:, 0:2, :n], msb[:, 1:3, :n]
                )
                ot = opool.tile([128, 2 * NT], mybir.dt.float32, tag="ot")
                otv = ot.rearrange("p (i j w) -> p j i w", j=2, w=W_SP)
                nc.vector.tensor_sub(
                    otv[:, :, :np_, :], st[:, :, :n], msb[:, 2:4, :n]
                )
                col0 = i0 * 2 * W_SP
                nc.sync.dma_start(
                    out[
                        img,
                        oc_t * 128 : (oc_t + 1) * 128,
                        col0 : col0 + 2 * n,
                    ],
                    ot[:, : 2 * n],
                )

            for img in range(B_PER):
                for blk in range(NBLK):
                    for oc_t in range(OC_TILES):
                        last = (
                            img == B_PER - 1
                            and blk == NBLK - 1
                            and oc_t == OC_TILES - 1
                        )
                        i0 = blk * PB
                        if last:
                            # split the final block so its first half's
                            # ACT/DVE/DMA chain overlaps the second half's
                            # matmuls (shortens the end-of-kernel drain)
                            process(img, i0, 4, oc_t, vts[img])
                            process(img, i0 + 4, 3, oc_t, vts[img])
                        else:
                            process(img, i0, PB, oc_t, vts[img])
                    if img == 0 and blk == 2:
                        v_ops(2, 0, NP, nc.vector)
                    if img == 1 and blk == 0:
                        v_ops(3, 0, NP, nc.vector)

    nc.compile()
    return nc


def _get_nc(mode: str):
    nc = _NC_CACHE.get(mode)
    if nc is None:
        nc = _build_nc(mode)
        _NC_CACHE[mode] = nc
    return nc


def kernel(x: np.ndarray, W: np.ndarray, b: np.ndarray) -> np.ndarray:
    mode = _mm_mode()
    x = np.asarray(x, dtype=np.float32)
    W = np.asarray(W, dtype=np.float32)
    b = np.asarray(b, dtype=np.float32)

    if mode == "bf16":
        import ml_dtypes

        in_np_dt = ml_dtypes.bfloat16
    elif mode == "f16":
        in_np_dt = np.float16
    else:
        in_np_dt = np.float32

    # Host-side layout prep: zero-pad x spatially; build the Winograd weight
    # taps U[t,kw] in lhsT layout [c, (t*KW+kw)*OC + oc]; stripe bias.
    xp = np.zeros((B, C, HP, WP), dtype=in_np_dt)
    xp[:, :, 1:-1, 1:-1] = x
    xp = xp.reshape(N_CORES, B_PER, C, HP * WP)

    g = W.reshape(OC, C, KH, KW)
    g0, g1, g2 = g[:, :, 0, :], g[:, :, 1, :], g[:, :, 2, :]
    u = np.stack(
        [g0, (g0 + g1 + g2) * 0.5, (g0 - g1 + g2) * 0.5, g2], axis=0
    )  # [t, OC, C, KW]
    wt = np.ascontiguousarray(
        u.transpose(2, 0, 3, 1).reshape(C, NTAP * KW * OC)
    ).astype(in_np_dt)
    bias = np.ascontiguousarray(b.reshape(OC_TILES, 128).T).astype(np.float32)

    nc = _get_nc(mode)
    in_maps = [
        {"xp": np.ascontiguousarray(xp[i]), "wt": wt, "bias": bias}
        for i in range(N_CORES)
    ]
    trace = os.environ.get("CONV_TRACE", "") not in ("", "0")
    try:
        res = bass_utils.run_bass_kernel_spmd(
            nc,
            in_maps,
            core_ids=list(range(N_CORES)),
            trace=trace,
        )
    except Exception:
        # transient device wedges (NRT_EXEC_UNIT_UNRECOVERABLE) have been
        # observed once; a fresh dispatch usually recovers
        import time

        time.sleep(2.0)
        res = bass_utils.run_bass_kernel_spmd(
            nc,
            in_maps,
            core_ids=list(range(N_CORES)),
            trace=trace,
        )
    kernel._last_results = res  # for test harness introspection
    out = np.stack([res.results[i]["out"] for i in range(N_CORES)])
    return out.reshape(B, OC, H, W_SP)


# revision 23
# speedup vs baseline: 1.3386x; 1.0025x over previous
"""Conv2D 3x3 (B=32, C=128, H=W=56 -> OC=256) as a Bass/Tile kernel on 8 NeuronCores.

Strategy: data-parallel over batch (4 images per core), W/b replicated,
1-D Winograd F(2,3) along H. Output row-pairs (2i, 2i+1) are produced from
4 transformed input taps:
  V0 = d0-d2, V1 = d1+d2, V2n = d1-d2 (= -V2), V3 = d1-d3
  (d_k = padded row 2i+k)
  M_t = sum_kw U[t,kw] @ V_t[:, :, kw:kw+56]     (PSUM, 12 matmuls/block
                                                  instead of direct conv's 18)
  y0 = M0 + M1 + M2 + b = m0 + (M1+b) - M2n
  y1 = M1 - M2 - M3 + b = (M1+b) + M2n - M3
U taps (host, from W):  U0=g0, U1=(g0+g1+g2)/2, U2=(g0-g1+g2)/2, U3=g2.

Per (img, 7-row-pair block, oc-half): 12 fp16 matmuls (N=392) accumulate
M0,M1,M2n,M3 into one 4-bank PSUM tile; ACT evacuates them to SBUF with the
bias folded into M1; DVE does 2 paired tensor_tensor passes
  [s0,s1] = [m0,m1b] + [m1b,m2n]   ;   [y0,y1] = [s0,s1] - [m2n,m3]
writing the final row-interleaved fp32 output tile, which DMAs out
contiguously. Input transform: DVE for images 0-1, GpSimd for images 2-3
(keeps DVE under the PE time). PE work drops from ~97us (direct) to ~65us.
"""

import os

import numpy as np

import concourse.bacc as bacc
import concourse.mybir as mybir
import concourse.tile as tile
from concourse import bass_utils

B, C, H, W_SP = 32, 128, 56, 56
OC, KH, KW = 256, 3, 3
N_CORES = 8
B_PER = B // N_CORES            # 4 images per core
HP, WP = H + 2, W_SP + 2        # zero-padded spatial dims (58x58)
HWO = H * W_SP                  # 3136
NP = H // 2                     # 28 output row-pairs
PB = 7                          # row-pairs per block
NBLK = NP // PB                 # 4 blocks per image
NT = PB * W_SP                  # 392 free columns per matmul
OC_TILES = OC // 128            # 2
NTAP = 4
# V tile tap order is (V0, V3, V1, V2n) so the paired DVE/GPS input-transform
# op can write V0/V3 contiguously; PSUM tap t -> V slot:
VMAP = [0, 2, 3, 1]             # t0->V0, t1->V1(slot2), t2->V2n(slot3), t3->V3(slot1)

_NC_CACHE: dict[str, object] = {}


def _mm_mode() -> str:
    return os.environ.get("CONV_MM_DTYPE", "f16")


def _build_nc(mode: str):
    in_dt = {
        "bf16": mybir.dt.bfloat16,
        "f16": mybir.dt.float16,
        "f32r": mybir.dt.float32r,
        "f32": mybir.dt.float32,
    }[mode]
    nc = bacc.Bacc(
        "TRN2",
        target_bir_lowering=False,
        debug=False,
        enable_asserts=False,
        num_devices=N_CORES,
    )
    xp = nc.dram_tensor("xp", [B_PER, C, HP * WP], in_dt, kind="ExternalInput").ap()
    wt = nc.dram_tensor(
        "wt", [C, NTAP * KW * OC], in_dt, kind="ExternalInput"
    ).ap()
    bias = nc.dram_tensor(
        "bias", [128, OC_TILES], mybir.dt.float32, kind="ExternalInput"
    ).ap()
    out = nc.dram_tensor(
        "out", [B_PER, OC, HWO], mybir.dt.float16, kind="ExternalOutput"
    ).ap()

    ident = mybir.ActivationFunctionType.Identity
    copyf = mybir.ActivationFunctionType.Copy

    with tile.TileContext(nc) as tc:
        with (
            tc.tile_pool(name="xin", bufs=4) as xpool,
            tc.tile_pool(name="vpool", bufs=4) as vpool,
            tc.tile_pool(name="wpool", bufs=1) as wpool,
            tc.tile_pool(name="bpool", bufs=1) as bpool,
            tc.tile_pool(name="mpool", bufs=5) as mpool,
            tc.tile_pool(name="spool", bufs=3) as spool,
            tc.tile_pool(name="opool", bufs=6) as opool,
            tc.tile_pool(name="psum", bufs=4, space="PSUM") as pspool,
        ):
            # HAM warm-up: burn matmuls on a zeroed tile while the lead-in
            # DMAs are in flight so the real MM stream starts at 2.4 GHz.
            # memset on DVE (its preamble finishes early; gpsimd's is late).
            wu = wpool.tile([C, 512], in_dt, tag="wu")
            nc.vector.memset(wu[:], 0.0)
            psw = pspool.tile([128, 2, 512], mybir.dt.float32, tag="ps")
            for i in range(8):
                nc.tensor.matmul(
                    psw[:, 0, :],
                    wu[:, :128],
                    wu[:],
                    start=(i == 0),
                    stop=(i == 7),
                )

            # lead-in DMAs. Sync HWDGE queue carries image 0 (quarters, so
            # its V chunks are ready just ahead of each block) and image 1
            # (halves); the GpSimd SWDGE queue runs in parallel with the
            # weights, bias and images 2,3.
            wsb = wpool.tile([C, NTAP * KW, OC], in_dt, tag="wsb")
            wtv = wt.rearrange("c (k m) -> c k m", m=OC)
            nc.gpsimd.dma_start(wsb[:, 0:KW, :], wtv[:, 0:KW, :])

            xts = []
            for img in range(B_PER):
                xts.append(
                    xpool.tile([C, HP, WP], in_dt, tag="xc", name=f"xc{img}")
                )
            xviews = [xp[img].rearrange("c (h w) -> c h w", w=WP) for img in range(B_PER)]
            # x0 quarter row-ranges with 2-row halos: V pair chunk k (7 pairs)
            # needs padded rows [14k, 14k+15]
            nc.sync.dma_start(xts[0][:, 0:17, :], xviews[0][:, 0:17, :])
            nc.gpsimd.dma_start(
                wsb[:, KW : NTAP * KW, :], wtv[:, KW : NTAP * KW, :]
            )
            nc.sync.dma_start(xts[0][:, 17:31, :], xviews[0][:, 17:31, :])
            nc.sync.dma_start(xts[0][:, 31:45, :], xviews[0][:, 31:45, :])
            nc.sync.dma_start(xts[0][:, 45:HP, :], xviews[0][:, 45:HP, :])
            nc.sync.dma_start(xts[1][:, 0:30, :], xviews[1][:, 0:30, :])
            nc.sync.dma_start(xts[1][:, 30:HP, :], xviews[1][:, 30:HP, :])

            bsb = bpool.tile([128, OC_TILES], mybir.dt.float32, tag="bsb")
            nc.gpsimd.dma_start(bsb[:], bias[:])
            nc.gpsimd.dma_start(xts[2][:], xviews[2][:])
            nc.gpsimd.dma_start(xts[3][:], xviews[3][:])

            # ---- input transform ----
            # V[c, slot, i, w], slots (V0, V3, V1, V2n).
            # xv2[c, a, i2, w] = x[c, 2*i2 + a, w]
            vts = []
            for img in range(B_PER):
                vts.append(
                    vpool.tile(
                        [C, NTAP, NP, WP], in_dt, tag="vt", name=f"vt{img}"
                    )
                )

            def v_ops(img, p0, np_, eng):
                # single-tap 2-free-dim ops: 3+ free dims fall off the DVE
                # 2x fast path (measured ~4x slower), singles hit it
                xv2 = xts[img].rearrange("c (i2 a) w -> c a i2 w", a=2)
                vt = vts[img]
                lo = slice(p0, p0 + np_)
                hi = slice(p0 + 1, p0 + np_ + 1)
                # V0 = d0 - d2
                eng.tensor_sub(vt[:, 0, lo, :], xv2[:, 0, lo, :], xv2[:, 0, hi, :])
                # V3 = d1 - d3
                eng.tensor_sub(vt[:, 1, lo, :], xv2[:, 1, lo, :], xv2[:, 1, hi, :])
                # V1 = d1 + d2
                eng.tensor_add(vt[:, 2, lo, :], xv2[:, 1, lo, :], xv2[:, 0, hi, :])
                # V2n = d1 - d2
                eng.tensor_sub(vt[:, 3, lo, :], xv2[:, 1, lo, :], xv2[:, 0, hi, :])

            # All input transforms on DVE: GpSimd tensor ops are ~4x slower,
            # pay a ~20us first-op IRAM load, and contend with DVE for the
            # shared SBUF port (measured quartering DVE throughput).
            # Image 0 chunked per quarter-DMA; image 1 per half; images 2,3
            # are emitted inside the block loop so the DVE FIFO reaches them
            # roughly when their x DMAs land.
            for q in range(4):
                v_ops(0, 7 * q, 7, nc.vector)
            v_ops(1, 0, 14, nc.vector)
            v_ops(1, 14, 14, nc.vector)

            # ---- main loop ----
            def process(img, i0, np_, oc_t, vt):
                n = np_ * W_SP
                # two 2-bank PSUM tiles per block: taps 0,1 in psa, 2,3 in
                # psb. Their single-tap evacs run as soon as each group
                # finishes, releasing banks early for the PE.
                psa = pspool.tile([128, 2, 512], mybir.dt.float32, tag="ps", name="psa")
                psb = pspool.tile([128, 2, 512], mybir.dt.float32, tag="ps", name="psb")
                for t in range(NTAP):
                    vs = VMAP[t]
                    ps = psa if t < 2 else psb
                    for kw in range(KW):
                        nc.tensor.matmul(
                            ps[:, t % 2, :n],
                            wsb[:, t * KW + kw, oc_t * 128 : (oc_t + 1) * 128],
                            vt[:, vs, i0 : i0 + np_, kw : kw + W_SP],
                            start=(kw == 0),
                            stop=(kw == KW - 1),
                        )
                msb = mpool.tile([128, NTAP, NT], in_dt, tag="m")
                # evacuate taps in group-completion order; bias rides M1
                nc.scalar.activation(msb[:, 0, :n], psa[:, 0, :n], copyf)
                nc.scalar.activation(
                    msb[:, 1, :n],
                    psa[:, 1, :n],
                    ident,
                    bias=bsb[:, oc_t : oc_t + 1],
                )
                nc.scalar.activation(msb[:, 2:4, :n], psb[:, 0:2, :n], copyf)
                st = spool.tile([128, 2, NT], in_dt, tag="s")
                nc.vector.tensor_add(
                    st[:, :, :n], msb[:, 0:2, :n], msb[:, 1:3, :n]
                )
                ot = opool.tile([128, 2 * NT], in_dt, tag="ot")
                otv = ot.rearrange("p (i j w) -> p j i w", j=2, w=W_SP)
                nc.vector.tensor_sub(
                    otv[:, :, :np_, :], st[:, :, :n], msb[:, 2:4, :n]
                )
                col0 = i0 * 2 * W_SP
                nc.sync.dma_start(
                    out[
                        img,
                        oc_t * 128 : (oc_t + 1) * 128,
                        col0 : col0 + 2 * n,
                    ],
                    ot[:, : 2 * n],
                )

            for img in range(B_PER):
                for blk in range(NBLK):
                    for oc_t in range(OC_TILES):
                        last = (
                            img == B_PER - 1
                            and blk == NBLK - 1
                            and oc_t == OC_TILES - 1
                        )
                        i0 = blk * PB
                        if last:
                            # split the final block so its first parts'
                            # ACT/DVE/DMA chains overlap the later parts'
                            # matmuls (shortens the end-of-kernel drain)
                            process(img, i0, 3, oc_t, vts[img])
                            process(img, i0 + 3, 2, oc_t, vts[img])
                            process(img, i0 + 5, 2, oc_t, vts[img])
                        else:
                            process(img, i0, PB, oc_t, vts[img])
                    if img == 0 and blk == 1:
                        v_ops(2, 0, NP, nc.vector)
                    if img == 0 and blk == 3:
                        v_ops(3, 0, NP, nc.vector)

    nc.compile()
    return nc


def _get_nc(mode: str):
    nc = _NC_CACHE.get(mode)
    if nc is None:
        nc = _build_nc(mode)
        _NC_CACHE[mode] = nc
    return nc


def kernel(x: np.ndarray, W: np.ndarray, b: np.ndarray) -> np.ndarray:
    mode = _mm_mode()
    x = np.asarray(x, dtype=np.float32)
    W = np.asarray(W, dtype=np.float32)
    b = np.asarray(b, dtype=np.float32)

    if mode == "bf16":
        import ml_dtypes

        in_np_dt = ml_dtypes.bfloat16
    elif mode == "f16":
        in_np_dt = np.float16
    else:
        in_np_dt = np.float32

    # Host-side layout prep: zero-pad x spatially; build the Winograd weight
    # taps U[t,kw] in lhsT layout [c, (t*KW+kw)*OC + oc]; stripe bias.
    xp = np.zeros((B, C, HP, WP), dtype=in_np_dt)
    xp[:, :, 1:-1, 1:-1] = x
    xp = xp.reshape(N_CORES, B_PER, C, HP * WP)

    g = W.reshape(OC, C, KH, KW)
    g0, g1, g2 = g[:, :, 0, :], g[:, :, 1, :], g[:, :, 2, :]
    u = np.stack(
        [g0, (g0 + g1 + g2) * 0.5, (g0 - g1 + g2) * 0.5, g2], axis=0
    )  # [t, OC, C, KW]
    wt = np.ascontiguousarray(
        u.transpose(2, 0, 3, 1).reshape(C, NTAP * KW * OC)
    ).astype(in_np_dt)
    bias = np.ascontiguousarray(b.reshape(OC_TILES, 128).T).astype(np.float32)

    nc = _get_nc(mode)
    in_maps = [
        {"xp": np.ascontiguousarray(xp[i]), "wt": wt, "bias": bias}
        for i in range(N_CORES)
    ]
    trace = os.environ.get("CONV_TRACE", "") not in ("", "0")
    try:
        res = bass_utils.run_bass_kernel_spmd(
            nc,
            in_maps,
            core_ids=list(range(N_CORES)),
            trace=trace,
        )
    except Exception:
        # transient device wedges (NRT_EXEC_UNIT_UNRECOVERABLE) have been
        # observed once; a fresh dispatch usually recovers
        import time

        time.sleep(2.0)
        res = bass_utils.run_bass_kernel_spmd(
            nc,
            in_maps,
            core_ids=list(range(N_CORES)),
            trace=trace,
        )
    kernel._last_results = res  # for test harness introspection
    out = np.stack([res.results[i]["out"] for i in range(N_CORES)])
    # device emits fp16; repack to the module's fp32 output dtype
    return out.reshape(B, OC, H, W_SP).astype(np.float32)


# revision 26
# speedup vs baseline: 1.3397x; 1.0009x over previous
"""Conv2D 3x3 (B=32, C=128, H=W=56 -> OC=256) as a Bass/Tile kernel on 8 NeuronCores.

Strategy: data-parallel over batch (4 images per core), W/b replicated,
1-D Winograd F(2,3) along H. Output row-pairs (2i, 2i+1) are produced from
4 transformed input taps:
  V0 = d0-d2, V1 = d1+d2, V2n = d1-d2 (= -V2), V3 = d1-d3
  (d_k = padded row 2i+k)
  M_t = sum_kw U[t,kw] @ V_t[:, :, kw:kw+56]     (PSUM, 12 matmuls/block
                                                  instead of direct conv's 18)
  y0 = M0 + M1 + M2 + b = m0 + (M1+b) - M2n
  y1 = M1 - M2 - M3 + b = (M1+b) + M2n - M3
U taps (host, from W):  U0=g0, U1=(g0+g1+g2)/2, U2=(g0-g1+g2)/2, U3=g2.

Per (img, 7-row-pair block, oc-half): 12 fp16 matmuls (N=392) accumulate
M0,M1,M2n,M3 into one 4-bank PSUM tile; ACT evacuates them to SBUF with the
bias folded into M1; DVE does 2 paired tensor_tensor passes
  [s0,s1] = [m0,m1b] + [m1b,m2n]   ;   [y0,y1] = [s0,s1] - [m2n,m3]
writing the final row-interleaved fp32 output tile, which DMAs out
contiguously. Input transform: DVE for images 0-1, GpSimd for images 2-3
(keeps DVE under the PE time). PE work drops from ~97us (direct) to ~65us.
"""

import os

import numpy as np

import concourse.bacc as bacc
import concourse.mybir as mybir
import concourse.tile as tile
from concourse import bass_utils

B, C, H, W_SP = 32, 128, 56, 56
OC, KH, KW = 256, 3, 3
N_CORES = 8
B_PER = B // N_CORES            # 4 images per core
HP, WP = H + 2, W_SP + 2        # zero-padded spatial dims (58x58)
HWO = H * W_SP                  # 3136
NP = H // 2                     # 28 output row-pairs
PB = 7                          # row-pairs per block
NBLK = NP // PB                 # 4 blocks per image
NT = PB * W_SP                  # 392 free columns per matmul
OC_TILES = OC // 128            # 2
NTAP = 4
# V tile tap order is (V0, V3, V1, V2n) so the paired DVE/GPS input-transform
# op can write V0/V3 contiguously; PSUM tap t -> V slot:
VMAP = [0, 2, 3, 1]             # t0->V0, t1->V1(slot2), t2->V2n(slot3), t3->V3(slot1)

_NC_CACHE: dict[str, object] = {}


def _mm_mode() -> str:
    return os.environ.get("CONV_MM_DTYPE", "f16")


def _build_nc(mode: str):
    in_dt = {
        "bf16": mybir.dt.bfloat16,
        "f16": mybir.dt.float16,
        "f32r": mybir.dt.float32r,
        "f32": mybir.dt.float32,
    }[mode]
    nc = bacc.Bacc(
        "TRN2",
        target_bir_lowering=False,
        debug=False,
        enable_asserts=False,
        num_devices=N_CORES,
    )
    xp = nc.dram_tensor("xp", [B_PER, C, HP * WP], in_dt, kind="ExternalInput").ap()
    wt = nc.dram_tensor(
        "wt", [C, NTAP * KW * OC], in_dt, kind="ExternalInput"
    ).ap()
    bias = nc.dram_tensor(
        "bias", [128, OC_TILES], mybir.dt.float32, kind="ExternalInput"
    ).ap()
    out = nc.dram_tensor(
        "out", [B_PER, OC, HWO], mybir.dt.float16, kind="ExternalOutput"
    ).ap()

    ident = mybir.ActivationFunctionType.Identity
    copyf = mybir.ActivationFunctionType.Copy

    with tile.TileContext(nc) as tc:
        with (
            tc.tile_pool(name="xin", bufs=4) as xpool,
            tc.tile_pool(name="vpool", bufs=4) as vpool,
            tc.tile_pool(name="wpool", bufs=1) as wpool,
            tc.tile_pool(name="bpool", bufs=1) as bpool,
            tc.tile_pool(name="mpool", bufs=5) as mpool,
            tc.tile_pool(name="spool", bufs=3) as spool,
            tc.tile_pool(name="opool", bufs=6) as opool,
            tc.tile_pool(name="psum", bufs=4, space="PSUM") as pspool,
        ):
            # HAM warm-up: burn matmuls on a zeroed tile while the lead-in
            # DMAs are in flight so the real MM stream starts at 2.4 GHz.
            # memset on DVE (its preamble finishes early; gpsimd's is late).
            wu = wpool.tile([C, 512], in_dt, tag="wu")
            nc.vector.memset(wu[:], 0.0)
            psw = pspool.tile([128, 2, 512], mybir.dt.float32, tag="ps")
            for i in range(8):
                nc.tensor.matmul(
                    psw[:, 0, :],
                    wu[:, :128],
                    wu[:],
                    start=(i == 0),
                    stop=(i == 7),
                )

            # lead-in DMAs. Sync HWDGE queue carries image 0 (quarters, so
            # its V chunks are ready just ahead of each block) and image 1
            # (halves); the GpSimd SWDGE queue runs in parallel with the
            # weights, bias and images 2,3.
            wsb = wpool.tile([C, NTAP * KW, OC], in_dt, tag="wsb")
            wtv = wt.rearrange("c (k m) -> c k m", m=OC)
            nc.gpsimd.dma_start(wsb[:, 0:KW, :], wtv[:, 0:KW, :])

            xviews = [xp[img].rearrange("c (h w) -> c h w", w=WP) for img in range(B_PER)]
            # x arrives as per-V-chunk tiles (pair chunk at p0, np pairs needs
            # padded rows [2*p0, 2*p0+2*np+1]) so each V op depends on exactly
            # one DMA — a whole-tile strided view would wait for all of them.
            # image 0 in quarters, image 1 in halves, images 2,3 whole.
            xchunks: dict[tuple[int, int], object] = {}

            def x_load(img, p0, np_, eng):
                r0, nr = 2 * p0, 2 * np_ + 2
                t = xpool.tile(
                    [C, nr, WP], in_dt, tag="xc", name=f"xc{img}_{p0}"
                )
                eng.dma_start(t[:], xviews[img][:, r0 : r0 + nr, :])
                xchunks[(img, p0)] = t

            x_load(0, 0, 7, nc.sync)
            nc.gpsimd.dma_start(
                wsb[:, KW : NTAP * KW, :], wtv[:, KW : NTAP * KW, :]
            )
            x_load(0, 7, 7, nc.sync)
            x_load(0, 14, 7, nc.sync)
            x_load(0, 21, 7, nc.sync)
            x_load(1, 0, 14, nc.sync)
            x_load(1, 14, 14, nc.sync)

            bsb = bpool.tile([128, OC_TILES], mybir.dt.float32, tag="bsb")
            nc.gpsimd.dma_start(bsb[:], bias[:])
            x_load(2, 0, NP, nc.gpsimd)
            x_load(3, 0, NP, nc.gpsimd)

            # ---- input transform ----
            # V[c, slot, i, w], slots (V0, V3, V1, V2n).
            # xv2[c, a, i2, w] = x[c, 2*i2 + a, w]
            vts = []
            for img in range(B_PER):
                vts.append(
                    vpool.tile(
                        [C, NTAP, NP, WP], in_dt, tag="vt", name=f"vt{img}"
                    )
                )

            def v_ops(img, p0, np_, eng):
                # single-tap 2-free-dim ops: 3+ free dims fall off the DVE
                # 2x fast path (measured ~4x slower), singles hit it
                xt = xchunks[(img, p0)]
                xv2 = xt[:, 0 : 2 * np_ + 2, :].rearrange(
                    "c (i2 a) w -> c a i2 w", a=2
                )
                vt = vts[img]
                lo = slice(p0, p0 + np_)
                li = slice(0, np_)
                hi = slice(1, np_ + 1)
                # V0 = d0 - d2
                eng.tensor_sub(vt[:, 0, lo, :], xv2[:, 0, li, :], xv2[:, 0, hi, :])
                # V3 = d1 - d3
                eng.tensor_sub(vt[:, 1, lo, :], xv2[:, 1, li, :], xv2[:, 1, hi, :])
                # V1 = d1 + d2
                eng.tensor_add(vt[:, 2, lo, :], xv2[:, 1, li, :], xv2[:, 0, hi, :])
                # V2n = d1 - d2
                eng.tensor_sub(vt[:, 3, lo, :], xv2[:, 1, li, :], xv2[:, 0, hi, :])

            # All input transforms on DVE: GpSimd tensor ops are ~4x slower,
            # pay a ~20us first-op IRAM load, and contend with DVE for the
            # shared SBUF port (measured quartering DVE throughput).
            # Image 0 chunked per quarter-DMA; image 1 per half; images 2,3
            # are emitted inside the block loop so the DVE FIFO reaches them
            # roughly when their x DMAs land.
            for q in range(4):
                v_ops(0, 7 * q, 7, nc.vector)
            v_ops(1, 0, 14, nc.vector)
            v_ops(1, 14, 14, nc.vector)

            # ---- main loop ----
            def process(img, i0, np_, oc_t, vt):
                n = np_ * W_SP
                # two 2-bank PSUM tiles per block: taps 0,1 in psa, 2,3 in
                # psb. Their single-tap evacs run as soon as each group
                # finishes, releasing banks early for the PE.
                psa = pspool.tile([128, 2, 512], mybir.dt.float32, tag="ps", name="psa")
                psb = pspool.tile([128, 2, 512], mybir.dt.float32, tag="ps", name="psb")
                for t in range(NTAP):
                    vs = VMAP[t]
                    ps = psa if t < 2 else psb
                    for kw in range(KW):
                        nc.tensor.matmul(
                            ps[:, t % 2, :n],
                            wsb[:, t * KW + kw, oc_t * 128 : (oc_t + 1) * 128],
                            vt[:, vs, i0 : i0 + np_, kw : kw + W_SP],
                            start=(kw == 0),
                            stop=(kw == KW - 1),
                        )
                msb = mpool.tile([128, NTAP, NT], in_dt, tag="m")
                # evacuate taps in group-completion order; bias rides M1
                nc.scalar.activation(msb[:, 0, :n], psa[:, 0, :n], copyf)
                nc.scalar.activation(
                    msb[:, 1, :n],
                    psa[:, 1, :n],
                    ident,
                    bias=bsb[:, oc_t : oc_t + 1],
                )
                nc.scalar.activation(msb[:, 2:4, :n], psb[:, 0:2, :n], copyf)
                st = spool.tile([128, 2, NT], in_dt, tag="s")
                nc.vector.tensor_add(
                    st[:, :, :n], msb[:, 0:2, :n], msb[:, 1:3, :n]
                )
                ot = opool.tile([128, 2 * NT], in_dt, tag="ot")
                otv = ot.rearrange("p (i j w) -> p j i w", j=2, w=W_SP)
                nc.vector.tensor_sub(
                    otv[:, :, :np_, :], st[:, :, :n], msb[:, 2:4, :n]
                )
                col0 = i0 * 2 * W_SP
                nc.sync.dma_start(
                    out[
                        img,
                        oc_t * 128 : (oc_t + 1) * 128,
                        col0 : col0 + 2 * n,
                    ],
                    ot[:, : 2 * n],
                )

            for img in range(B_PER):
                for blk in range(NBLK):
                    for oc_t in range(OC_TILES):
                        last = (
                            img == B_PER - 1
                            and blk == NBLK - 1
                            and oc_t == OC_TILES - 1
                        )
                        i0 = blk * PB
                        if last:
                            # split the final block so its first half's
                            # ACT/DVE/DMA chain overlaps the second half's
                            # matmuls (shortens the end-of-kernel drain)
                            process(img, i0, 4, oc_t, vts[img])
                            process(img, i0 + 4, 3, oc_t, vts[img])
                        else:
                            process(img, i0, PB, oc_t, vts[img])
                    if img == 0 and blk == 1:
                        v_ops(2, 0, NP, nc.vector)
                    if img == 0 and blk == 3:
                        v_ops(3, 0, NP, nc.vector)

    nc.compile()
    return nc


def _get_nc(mode: str):
    nc = _NC_CACHE.get(mode)
    if nc is None:
        nc = _build_nc(mode)
        _NC_CACHE[mode] = nc
    return nc


def kernel(x: np.ndarray, W: np.ndarray, b: np.ndarray) -> np.ndarray:
    mode = _mm_mode()
    x = np.asarray(x, dtype=np.float32)
    W = np.asarray(W, dtype=np.float32)
    b = np.asarray(b, dtype=np.float32)

    if mode == "bf16":
        import ml_dtypes

        in_np_dt = ml_dtypes.bfloat16
    elif mode == "f16":
        in_np_dt = np.float16
    else:
        in_np_dt = np.float32

    # Host-side layout prep: zero-pad x spatially; build the Winograd weight
    # taps U[t,kw] in lhsT layout [c, (t*KW+kw)*OC + oc]; stripe bias.
    xp = np.zeros((B, C, HP, WP), dtype=in_np_dt)
    xp[:, :, 1:-1, 1:-1] = x
    xp = xp.reshape(N_CORES, B_PER, C, HP * WP)

    g = W.reshape(OC, C, KH, KW)
    g0, g1, g2 = g[:, :, 0, :], g[:, :, 1, :], g[:, :, 2, :]
    u = np.stack(
        [g0, (g0 + g1 + g2) * 0.5, (g0 - g1 + g2) * 0.5, g2], axis=0
    )  # [t, OC, C, KW]
    wt = np.ascontiguousarray(
        u.transpose(2, 0, 3, 1).reshape(C, NTAP * KW * OC)
    ).astype(in_np_dt)
    bias = np.ascontiguousarray(b.reshape(OC_TILES, 128).T).astype(np.float32)

    nc = _get_nc(mode)
    in_maps = [
        {"xp": np.ascontiguousarray(xp[i]), "wt": wt, "bias": bias}
        for i in range(N_CORES)
    ]
    trace = os.environ.get("CONV_TRACE", "") not in ("", "0")
    try:
        res = bass_utils.run_bass_kernel_spmd(
            nc,
            in_maps,
            core_ids=list(range(N_CORES)),
            trace=trace,
        )
    except Exception:
        # transient device wedges (NRT_EXEC_UNIT_UNRECOVERABLE) have been
        # observed once; a fresh dispatch usually recovers
        import time

        time.sleep(2.0)
        res = bass_utils.run_bass_kernel_spmd(
            nc,
            in_maps,
            core_ids=list(range(N_CORES)),
            trace=trace,
        )
    kernel._last_results = res  # for test harness introspection
    out = np.stack([res.results[i]["out"] for i in range(N_CORES)])
    # device emits fp16; repack to the module's fp32 output dtype
    return out.reshape(B, OC, H, W_SP).astype(np.float32)


# revision 29
# speedup vs baseline: 1.3442x; 1.0033x over previous
"""Conv2D 3x3 (B=32, C=128, H=W=56 -> OC=256) as a Bass/Tile kernel on 8 NeuronCores.

Strategy: data-parallel over batch (4 images per core), W/b replicated,
1-D Winograd F(2,3) along H. Output row-pairs (2i, 2i+1) are produced from
4 transformed input taps:
  V0 = d0-d2, V1 = d1+d2, V2n = d1-d2 (= -V2), V3 = d1-d3
  (d_k = padded row 2i+k)
  M_t = sum_kw U[t,kw] @ V_t[:, :, kw:kw+56]     (PSUM, 12 matmuls/block
                                                  instead of direct conv's 18)
  y0 = M0 + M1 + M2 + b = m0 + (M1+b) - M2n
  y1 = M1 - M2 - M3 + b = (M1+b) + M2n - M3
U taps (host, from W):  U0=g0, U1=(g0+g1+g2)/2, U2=(g0-g1+g2)/2, U3=g2.

Per (img, 7-row-pair block, oc-half): 12 fp16 matmuls (N=392) accumulate
M0,M1,M2n,M3 into one 4-bank PSUM tile; ACT evacuates them to SBUF with the
bias folded into M1; DVE does 2 paired tensor_tensor passes
  [s0,s1] = [m0,m1b] + [m1b,m2n]   ;   [y0,y1] = [s0,s1] - [m2n,m3]
writing the final row-interleaved fp32 output tile, which DMAs out
contiguously. Input transform: DVE for images 0-1, GpSimd for images 2-3
(keeps DVE under the PE time). PE work drops from ~97us (direct) to ~65us.
"""

import os

import numpy as np

import concourse.bacc as bacc
import concourse.mybir as mybir
import concourse.tile as tile
from concourse import bass_utils

B, C, H, W_SP = 32, 128, 56, 56
OC, KH, KW = 256, 3, 3
N_CORES = 8
B_PER = B // N_CORES            # 4 images per core
HP, WP = H + 2, W_SP + 2        # zero-padded spatial dims (58x58)
HWO = H * W_SP                  # 3136
NP = H // 2                     # 28 output row-pairs
PB = 7                          # row-pairs per block
NBLK = NP // PB                 # 4 blocks per image
NT = PB * W_SP                  # 392 free columns per matmul
OC_TILES = OC // 128            # 2
NTAP = 4
# V tile tap order is (V0, V3, V1, V2n) so the paired DVE/GPS input-transform
# op can write V0/V3 contiguously; PSUM tap t -> V slot:
VMAP = [0, 2, 3, 1]             # t0->V0, t1->V1(slot2), t2->V2n(slot3), t3->V3(slot1)

_NC_CACHE: dict[str, object] = {}


def _mm_mode() -> str:
    return os.environ.get("CONV_MM_DTYPE", "f16")


def _build_nc(mode: str):
    in_dt = {
        "bf16": mybir.dt.bfloat16,
        "f16": mybir.dt.float16,
        "f32r": mybir.dt.float32r,
        "f32": mybir.dt.float32,
    }[mode]
    nc = bacc.Bacc(
        "TRN2",
        target_bir_lowering=False,
        debug=False,
        enable_asserts=False,
        num_devices=N_CORES,
    )
    xp = nc.dram_tensor("xp", [B_PER, C, HP * WP], in_dt, kind="ExternalInput").ap()
    wt = nc.dram_tensor(
        "wt", [C, NTAP * KW * OC], in_dt, kind="ExternalInput"
    ).ap()
    bias = nc.dram_tensor(
        "bias", [128, OC_TILES], mybir.dt.float32, kind="ExternalInput"
    ).ap()
    out = nc.dram_tensor(
        "out", [B_PER, OC, HWO], mybir.dt.float16, kind="ExternalOutput"
    ).ap()

    ident = mybir.ActivationFunctionType.Identity
    copyf = mybir.ActivationFunctionType.Copy

    with tile.TileContext(nc) as tc:
        with (
            tc.tile_pool(name="xin", bufs=8) as xpool,
            tc.tile_pool(name="vpool", bufs=4) as vpool,
            tc.tile_pool(name="wpool", bufs=1) as wpool,
            tc.tile_pool(name="bpool", bufs=1) as bpool,
            tc.tile_pool(name="mpool", bufs=5) as mpool,
            tc.tile_pool(name="spool", bufs=3) as spool,
            tc.tile_pool(name="opool", bufs=6) as opool,
            tc.tile_pool(name="psum", bufs=4, space="PSUM") as pspool,
        ):
            # HAM warm-up: burn matmuls on a zeroed tile while the lead-in
            # DMAs are in flight so the real MM stream starts at 2.4 GHz.
            # memset on DVE (its preamble finishes early; gpsimd's is late).
            wu = wpool.tile([C, 512], in_dt, tag="wu")
            nc.vector.memset(wu[:], 0.0)
            psw = pspool.tile([128, 2, 512], mybir.dt.float32, tag="ps")
            for i in range(8):
                nc.tensor.matmul(
                    psw[:, 0, :],
                    wu[:, :128],
                    wu[:],
                    start=(i == 0),
                    stop=(i == 7),
                )

            # lead-in DMAs. Sync HWDGE queue carries image 0 (quarters, so
            # its V chunks are ready just ahead of each block) and image 1
            # (halves); the GpSimd SWDGE queue runs in parallel with the
            # weights, bias and images 2,3.
            wsb = wpool.tile([C, NTAP * KW, OC], in_dt, tag="wsb")
            wtv = wt.rearrange("c (k m) -> c k m", m=OC)
            nc.gpsimd.dma_start(wsb[:, 0:KW, :], wtv[:, 0:KW, :])

            xviews = [xp[img].rearrange("c (h w) -> c h w", w=WP) for img in range(B_PER)]
            # x arrives as per-V-chunk tiles (pair chunk at p0, np pairs needs
            # padded rows [2*p0, 2*p0+2*np+1]) so each V op depends on exactly
            # one DMA — a whole-tile strided view would wait for all of them.
            # image 0 in quarters, image 1 in halves, images 2,3 whole.
            xchunks: dict[tuple[int, int], object] = {}

            def x_load(img, p0, np_, eng):
                r0, nr = 2 * p0, 2 * np_ + 2
                t = xpool.tile(
                    [C, nr, WP], in_dt, tag="xc", name=f"xc{img}_{p0}"
                )
                eng.dma_start(t[:], xviews[img][:, r0 : r0 + nr, :])
                xchunks[(img, p0)] = t

            x_load(0, 0, 7, nc.sync)
            nc.gpsimd.dma_start(
                wsb[:, KW : NTAP * KW, :], wtv[:, KW : NTAP * KW, :]
            )
            x_load(0, 7, 7, nc.sync)
            x_load(0, 14, 7, nc.sync)
            x_load(0, 21, 7, nc.sync)
            x_load(1, 0, 14, nc.sync)
            x_load(1, 14, 14, nc.sync)

            bsb = bpool.tile([128, OC_TILES], mybir.dt.float32, tag="bsb")
            nc.gpsimd.dma_start(bsb[:], bias[:])
            x_load(2, 0, NP, nc.gpsimd)
            x_load(3, 0, NP, nc.gpsimd)

            # ---- input transform ----
            # V[c, slot, i, w], slots (V0, V3, V1, V2n).
            # xv2[c, a, i2, w] = x[c, 2*i2 + a, w]
            vts = []
            for img in range(B_PER):
                vts.append(
                    vpool.tile(
                        [C, NTAP, NP, WP], in_dt, tag="vt", name=f"vt{img}"
                    )
                )

            def v_ops(img, p0, np_, eng):
                # single-tap 2-free-dim ops: 3+ free dims fall off the DVE
                # 2x fast path (measured ~4x slower), singles hit it
                xt = xchunks[(img, p0)]
                xv2 = xt[:, 0 : 2 * np_ + 2, :].rearrange(
                    "c (i2 a) w -> c a i2 w", a=2
                )
                vt = vts[img]
                lo = slice(p0, p0 + np_)
                li = slice(0, np_)
                hi = slice(1, np_ + 1)
                # V0 = d0 - d2
                eng.tensor_sub(vt[:, 0, lo, :], xv2[:, 0, li, :], xv2[:, 0, hi, :])
                # V3 = d1 - d3
                eng.tensor_sub(vt[:, 1, lo, :], xv2[:, 1, li, :], xv2[:, 1, hi, :])
                # V1 = d1 + d2
                eng.tensor_add(vt[:, 2, lo, :], xv2[:, 1, li, :], xv2[:, 0, hi, :])
                # V2n = d1 - d2
                eng.tensor_sub(vt[:, 3, lo, :], xv2[:, 1, li, :], xv2[:, 0, hi, :])

            # All input transforms on DVE: GpSimd tensor ops are ~4x slower,
            # pay a ~20us first-op IRAM load, and contend with DVE for the
            # shared SBUF port (measured quartering DVE throughput).
            # IMPORTANT: Tile coalesces semaphore waits toward the most
            # recently emitted producer, so V chunks must be emitted
            # interleaved with the blocks that consume them (one chunk
            # ahead), never all up front — otherwise block 0 waits on the
            # last V op.
            v_ops(0, 0, 7, nc.vector)
            v_ops(0, 7, 7, nc.vector)
            # (chunk k+2 / later images are emitted inside the block loop)
            v_sched = {
                (0, 0): [(0, 14, 7)],
                (0, 1): [(0, 21, 7)],
                (0, 2): [(1, 0, 14)],
                (0, 3): [(1, 14, 14)],
                (1, 0): [(2, 0, NP)],
                (1, 2): [(3, 0, NP)],
            }

            # ---- main loop ----
            def process(img, i0, np_, oc_t, vt):
                n = np_ * W_SP
                # two 2-bank PSUM tiles per block: taps 0,1 in psa, 2,3 in
                # psb. Their single-tap evacs run as soon as each group
                # finishes, releasing banks early for the PE.
                psa = pspool.tile([128, 2, 512], mybir.dt.float32, tag="ps", name="psa")
                psb = pspool.tile([128, 2, 512], mybir.dt.float32, tag="ps", name="psb")
                for t in range(NTAP):
                    vs = VMAP[t]
                    ps = psa if t < 2 else psb
                    for kw in range(KW):
                        nc.tensor.matmul(
                            ps[:, t % 2, :n],
                            wsb[:, t * KW + kw, oc_t * 128 : (oc_t + 1) * 128],
                            vt[:, vs, i0 : i0 + np_, kw : kw + W_SP],
                            start=(kw == 0),
                            stop=(kw == KW - 1),
                        )
                msb = mpool.tile([128, NTAP, NT], in_dt, tag="m")
                # evacuate taps in group-completion order; bias rides M1
                nc.scalar.activation(msb[:, 0, :n], psa[:, 0, :n], copyf)
                nc.scalar.activation(
                    msb[:, 1, :n],
                    psa[:, 1, :n],
                    ident,
                    bias=bsb[:, oc_t : oc_t + 1],
                )
                nc.scalar.activation(msb[:, 2:4, :n], psb[:, 0:2, :n], copyf)
                st = spool.tile([128, 2, NT], in_dt, tag="s")
                nc.vector.tensor_add(
                    st[:, :, :n], msb[:, 0:2, :n], msb[:, 1:3, :n]
                )
                ot = opool.tile([128, 2 * NT], in_dt, tag="ot")
                otv = ot.rearrange("p (i j w) -> p j i w", j=2, w=W_SP)
                nc.vector.tensor_sub(
                    otv[:, :, :np_, :], st[:, :, :n], msb[:, 2:4, :n]
                )
                col0 = i0 * 2 * W_SP
                nc.sync.dma_start(
                    out[
                        img,
                        oc_t * 128 : (oc_t + 1) * 128,
                        col0 : col0 + 2 * n,
                    ],
                    ot[:, : 2 * n],
                )

            for img in range(B_PER):
                for blk in range(NBLK):
                    for oc_t in range(OC_TILES):
                        last = (
                            img == B_PER - 1
                            and blk == NBLK - 1
                            and oc_t == OC_TILES - 1
                        )
                        i0 = blk * PB
                        if last:
                            # split the final block so its first half's
                            # ACT/DVE/DMA chain overlaps the second half's
                            # matmuls (shortens the end-of-kernel drain)
                            process(img, i0, 4, oc_t, vts[img])
                            process(img, i0 + 4, 3, oc_t, vts[img])
                        else:
                            process(img, i0, PB, oc_t, vts[img])
                    for vimg, vp0, vnp in v_sched.get((img, blk), ()):
                        v_ops(vimg, vp0, vnp, nc.vector)

    nc.compile()
    return nc


def _get_nc(mode: str):
    nc = _NC_CACHE.get(mode)
    if nc is None:
        nc = _build_nc(mode)
        _NC_CACHE[mode] = nc
    return nc


def kernel(x: np.ndarray, W: np.ndarray, b: np.ndarray) -> np.ndarray:
    mode = _mm_mode()
    x = np.asarray(x, dtype=np.float32)
    W = np.asarray(W, dtype=np.float32)
    b = np.asarray(b, dtype=np.float32)

    if mode == "bf16":
        import ml_dtypes

        in_np_dt = ml_dtypes.bfloat16
    elif mode == "f16":
        in_np_dt = np.float16
    else:
        in_np_dt = np.float32

    # Host-side layout prep: zero-pad x spatially; build the Winograd weight
    # taps U[t,kw] in lhsT layout [c, (t*KW+kw)*OC + oc]; stripe bias.
    xp = np.zeros((B, C, HP, WP), dtype=in_np_dt)
    xp[:, :, 1:-1, 1:-1] = x
    xp = xp.reshape(N_CORES, B_PER, C, HP * WP)

    g = W.reshape(OC, C, KH, KW)
    g0, g1, g2 = g[:, :, 0, :], g[:, :, 1, :], g[:, :, 2, :]
    u = np.stack(
        [g0, (g0 + g1 + g2) * 0.5, (g0 - g1 + g2) * 0.5, g2], axis=0
    )  # [t, OC, C, KW]
    wt = np.ascontiguousarray(
        u.transpose(2, 0, 3, 1).reshape(C, NTAP * KW * OC)
    ).astype(in_np_dt)
    bias = np.ascontiguousarray(b.reshape(OC_TILES, 128).T).astype(np.float32)

    nc = _get_nc(mode)
    in_maps = [
        {"xp": np.ascontiguousarray(xp[i]), "wt": wt, "bias": bias}
        for i in range(N_CORES)
    ]
    trace = os.environ.get("CONV_TRACE", "") not in ("", "0")
    try:
        res = bass_utils.run_bass_kernel_spmd(
            nc,
            in_maps,
            core_ids=list(range(N_CORES)),
            trace=trace,
        )
    except Exception:
        # transient device wedges (NRT_EXEC_UNIT_UNRECOVERABLE) have been
        # observed once; a fresh dispatch usually recovers
        import time

        time.sleep(2.0)
        res = bass_utils.run_bass_kernel_spmd(
            nc,
            in_maps,
            core_ids=list(range(N_CORES)),
            trace=trace,
        )
    kernel._last_results = res  # for test harness introspection
    out = np.stack([res.results[i]["out"] for i in range(N_CORES)])
    # device emits fp16; repack to the module's fp32 output dtype
    return out.reshape(B, OC, H, W_SP).astype(np.float32)


# revision 32
# speedup vs baseline: 1.3491x; 1.0036x over previous
"""Conv2D 3x3 (B=32, C=128, H=W=56 -> OC=256) as a Bass/Tile kernel on 8 NeuronCores.

Strategy: data-parallel over batch (4 images per core), W/b replicated,
1-D Winograd F(2,3) along H. Output row-pairs (2i, 2i+1) are produced from
4 transformed input taps:
  V0 = d0-d2, V1 = d1+d2, V2n = d1-d2 (= -V2), V3 = d1-d3
  (d_k = padded row 2i+k)
  M_t = sum_kw U[t,kw] @ V_t[:, :, kw:kw+56]     (PSUM, 12 matmuls/block
                                                  instead of direct conv's 18)
  y0 = M0 + M1 + M2 + b = m0 + (M1+b) - M2n
  y1 = M1 - M2 - M3 + b = (M1+b) + M2n - M3
U taps (host, from W):  U0=g0, U1=(g0+g1+g2)/2, U2=(g0-g1+g2)/2, U3=g2.

Per (img, 7-row-pair block, oc-half): 12 fp16 matmuls (N=392) accumulate
M0,M1,M2n,M3 into one 4-bank PSUM tile; ACT evacuates them to SBUF with the
bias folded into M1; DVE does 2 paired tensor_tensor passes
  [s0,s1] = [m0,m1b] + [m1b,m2n]   ;   [y0,y1] = [s0,s1] - [m2n,m3]
writing the final row-interleaved fp32 output tile, which DMAs out
contiguously. Input transform: DVE for images 0-1, GpSimd for images 2-3
(keeps DVE under the PE time). PE work drops from ~97us (direct) to ~65us.
"""

import os

import numpy as np

import concourse.bacc as bacc
import concourse.mybir as mybir
import concourse.tile as tile
from concourse import bass_utils

B, C, H, W_SP = 32, 128, 56, 56
OC, KH, KW = 256, 3, 3
N_CORES = 8
B_PER = B // N_CORES            # 4 images per core
HP, WP = H + 2, W_SP + 2        # zero-padded spatial dims (58x58)
HWO = H * W_SP                  # 3136
NP = H // 2                     # 28 output row-pairs
PB = 7                          # row-pairs per block
NBLK = NP // PB                 # 4 blocks per image
NT = PB * W_SP                  # 392 free columns per matmul
OC_TILES = OC // 128            # 2
NTAP = 4
# V tile tap order is (V0, V3, V1, V2n) so the paired DVE/GPS input-transform
# op can write V0/V3 contiguously; PSUM tap t -> V slot:
VMAP = [0, 2, 3, 1]             # t0->V0, t1->V1(slot2), t2->V2n(slot3), t3->V3(slot1)

_NC_CACHE: dict[str, object] = {}


def _mm_mode() -> str:
    return os.environ.get("CONV_MM_DTYPE", "f16")


def _build_nc(mode: str):
    in_dt = {
        "bf16": mybir.dt.bfloat16,
        "f16": mybir.dt.float16,
        "f32r": mybir.dt.float32r,
        "f32": mybir.dt.float32,
    }[mode]
    nc = bacc.Bacc(
        "TRN2",
        target_bir_lowering=False,
        debug=False,
        enable_asserts=False,
        num_devices=N_CORES,
    )
    xp = nc.dram_tensor("xp", [B_PER, C, HP * WP], in_dt, kind="ExternalInput").ap()
    wt = nc.dram_tensor(
        "wt", [C, NTAP * KW * OC], in_dt, kind="ExternalInput"
    ).ap()
    bias = nc.dram_tensor(
        "bias", [128, OC_TILES], mybir.dt.float32, kind="ExternalInput"
    ).ap()
    out = nc.dram_tensor(
        "out", [B_PER, OC, HWO], mybir.dt.float16, kind="ExternalOutput"
    ).ap()

    ident = mybir.ActivationFunctionType.Identity
    copyf = mybir.ActivationFunctionType.Copy

    with tile.TileContext(nc) as tc:
        with (
            tc.tile_pool(name="xin", bufs=8) as xpool,
            tc.tile_pool(name="vpool", bufs=4) as vpool,
            tc.tile_pool(name="wpool", bufs=1) as wpool,
            tc.tile_pool(name="bpool", bufs=1) as bpool,
            tc.tile_pool(name="mpool", bufs=5) as mpool,
            tc.tile_pool(name="spool", bufs=3) as spool,
            tc.tile_pool(name="opool", bufs=6) as opool,
            tc.tile_pool(name="psum", bufs=4, space="PSUM") as pspool,
        ):
            # HAM warm-up: burn matmuls on a zeroed tile while the lead-in
            # DMAs are in flight so the real MM stream starts at 2.4 GHz.
            # memset on DVE (its preamble finishes early; gpsimd's is late).
            wu = wpool.tile([C, 512], in_dt, tag="wu")
            nc.vector.memset(wu[:], 0.0)
            psw = pspool.tile([128, 2, 512], mybir.dt.float32, tag="ps")
            for i in range(8):
                nc.tensor.matmul(
                    psw[:, 0, :],
                    wu[:, :128],
                    wu[:],
                    start=(i == 0),
                    stop=(i == 7),
                )

            # lead-in DMAs. Sync HWDGE queue carries image 0 (quarters, so
            # its V chunks are ready just ahead of each block) and image 1
            # (halves); the GpSimd SWDGE queue runs in parallel with the
            # weights, bias and images 2,3.
            wsb = wpool.tile([C, NTAP * KW, OC], in_dt, tag="wsb")
            wtv = wt.rearrange("c (k m) -> c k m", m=OC)
            nc.gpsimd.dma_start(wsb[:, 0:KW, :], wtv[:, 0:KW, :])

            xviews = [xp[img].rearrange("c (h w) -> c h w", w=WP) for img in range(B_PER)]
            # x arrives as per-V-chunk tiles (pair chunk at p0, np pairs needs
            # padded rows [2*p0, 2*p0+2*np+1]) so each V op depends on exactly
            # one DMA — a whole-tile strided view would wait for all of them.
            # image 0 in quarters, image 1 in halves, images 2,3 whole.
            xchunks: dict[tuple[int, int], object] = {}

            def x_load(img, p0, np_, eng):
                r0, nr = 2 * p0, 2 * np_ + 2
                t = xpool.tile(
                    [C, nr, WP], in_dt, tag="xc", name=f"xc{img}_{p0}"
                )
                eng.dma_start(t[:], xviews[img][:, r0 : r0 + nr, :])
                xchunks[(img, p0)] = t

            x_load(0, 0, 7, nc.sync)
            nc.gpsimd.dma_start(
                wsb[:, KW : NTAP * KW, :], wtv[:, KW : NTAP * KW, :]
            )
            x_load(0, 7, 7, nc.sync)
            bsb = bpool.tile([128, OC_TILES], mybir.dt.float32, tag="bsb")
            nc.gpsimd.dma_start(bsb[:], bias[:])
            # remaining x chunks are emitted inside the block loop (see
            # v_sched) — DMA-completion waits also coalesce to the latest
            # emitted DMA on the queue, so early consumers must not have
            # later loads emitted ahead of them

            # ---- input transform ----
            # V[c, slot, i, w], slots (V0, V3, V1, V2n).
            # xv2[c, a, i2, w] = x[c, 2*i2 + a, w]
            vts = []
            for img in range(B_PER):
                vts.append(
                    vpool.tile(
                        [C, NTAP, NP, WP], in_dt, tag="vt", name=f"vt{img}"
                    )
                )

            def v_ops(img, p0, np_, eng):
                # single-tap 2-free-dim ops: 3+ free dims fall off the DVE
                # 2x fast path (measured ~4x slower), singles hit it
                xt = xchunks[(img, p0)]
                xv2 = xt[:, 0 : 2 * np_ + 2, :].rearrange(
                    "c (i2 a) w -> c a i2 w", a=2
                )
                vt = vts[img]
                lo = slice(p0, p0 + np_)
                li = slice(0, np_)
                hi = slice(1, np_ + 1)
                # V0 = d0 - d2
                eng.tensor_sub(vt[:, 0, lo, :], xv2[:, 0, li, :], xv2[:, 0, hi, :])
                # V3 = d1 - d3
                eng.tensor_sub(vt[:, 1, lo, :], xv2[:, 1, li, :], xv2[:, 1, hi, :])
                # V1 = d1 + d2
                eng.tensor_add(vt[:, 2, lo, :], xv2[:, 1, li, :], xv2[:, 0, hi, :])
                # V2n = d1 - d2
                eng.tensor_sub(vt[:, 3, lo, :], xv2[:, 1, li, :], xv2[:, 0, hi, :])

            # All input transforms on DVE: GpSimd tensor ops are ~4x slower,
            # pay a ~20us first-op IRAM load, and contend with DVE for the
            # shared SBUF port (measured quartering DVE throughput).
            # IMPORTANT: Tile coalesces semaphore waits toward the most
            # recently emitted producer, so V chunks must be emitted
            # interleaved with the blocks that consume them (one chunk
            # ahead), never all up front — otherwise block 0 waits on the
            # last V op.
            v_ops(0, 0, 7, nc.vector)
            # per-(img, blk) actions emitted after that block's processes:
            # x loads first (queue, p0, np), then v chunk transforms
            v_sched = {
                (0, 0): ([("s", 0, 14, 7)], [(0, 7, 7)]),
                (0, 1): ([("s", 0, 21, 7), ("g", 2, 0, NP)], [(0, 14, 7)]),
                (0, 2): ([("s", 1, 0, 14), ("g", 3, 0, NP)], [(0, 21, 7)]),
                (0, 3): ([("s", 1, 14, 14)], [(1, 0, 14)]),
                (1, 0): ((), [(1, 14, 14)]),
                (1, 1): ((), [(2, 0, NP)]),
                (1, 3): ((), [(3, 0, NP)]),
            }

            # ---- main loop ----
            def process(img, i0, np_, oc_t, vt):
                n = np_ * W_SP
                # two 2-bank PSUM tiles per block: taps 0,1 in psa, 2,3 in
                # psb. Their single-tap evacs run as soon as each group
                # finishes, releasing banks early for the PE.
                psa = pspool.tile([128, 2, 512], mybir.dt.float32, tag="ps", name="psa")
                psb = pspool.tile([128, 2, 512], mybir.dt.float32, tag="ps", name="psb")
                for t in range(NTAP):
                    vs = VMAP[t]
                    ps = psa if t < 2 else psb
                    for kw in range(KW):
                        nc.tensor.matmul(
                            ps[:, t % 2, :n],
                            wsb[:, t * KW + kw, oc_t * 128 : (oc_t + 1) * 128],
                            vt[:, vs, i0 : i0 + np_, kw : kw + W_SP],
                            start=(kw == 0),
                            stop=(kw == KW - 1),
                        )
                msb = mpool.tile([128, NTAP, NT], in_dt, tag="m")
                # evacuate taps in group-completion order; bias rides M1
                nc.scalar.activation(msb[:, 0, :n], psa[:, 0, :n], copyf)
                nc.scalar.activation(
                    msb[:, 1, :n],
                    psa[:, 1, :n],
                    ident,
                    bias=bsb[:, oc_t : oc_t + 1],
                )
                nc.scalar.activation(msb[:, 2:4, :n], psb[:, 0:2, :n], copyf)
                st = spool.tile([128, 2, NT], in_dt, tag="s")
                nc.vector.tensor_add(
                    st[:, :, :n], msb[:, 0:2, :n], msb[:, 1:3, :n]
                )
                ot = opool.tile([128, 2 * NT], in_dt, tag="ot")
                otv = ot.rearrange("p (i j w) -> p j i w", j=2, w=W_SP)
                nc.vector.tensor_sub(
                    otv[:, :, :np_, :], st[:, :, :n], msb[:, 2:4, :n]
                )
                col0 = i0 * 2 * W_SP
                nc.sync.dma_start(
                    out[
                        img,
                        oc_t * 128 : (oc_t + 1) * 128,
                        col0 : col0 + 2 * n,
                    ],
                    ot[:, : 2 * n],
                )

            for img in range(B_PER):
                for blk in range(NBLK):
                    for oc_t in range(OC_TILES):
                        last = (
                            img == B_PER - 1
                            and blk == NBLK - 1
                            and oc_t == OC_TILES - 1
                        )
                        i0 = blk * PB
                        if last:
                            # split the final block so its first half's
                            # ACT/DVE/DMA chain overlaps the second half's
                            # matmuls (shortens the end-of-kernel drain)
                            process(img, i0, 4, oc_t, vts[img])
                            process(img, i0 + 4, 3, oc_t, vts[img])
                        else:
                            process(img, i0, PB, oc_t, vts[img])
                    xl, vl = v_sched.get((img, blk), ((), ()))
                    for q, ximg, xp0, xnp in xl:
                        x_load(
                            ximg, xp0, xnp,
                            nc.sync if q == "s" else nc.gpsimd,
                        )
                    for vimg, vp0, vnp in vl:
                        v_ops(vimg, vp0, vnp, nc.vector)

    nc.compile()
    return nc


def _get_nc(mode: str):
    nc = _NC_CACHE.get(mode)
    if nc is None:
        nc = _build_nc(mode)
        _NC_CACHE[mode] = nc
    return nc


def kernel(x: np.ndarray, W: np.ndarray, b: np.ndarray) -> np.ndarray:
    mode = _mm_mode()
    x = np.asarray(x, dtype=np.float32)
    W = np.asarray(W, dtype=np.float32)
    b = np.asarray(b, dtype=np.float32)

    if mode == "bf16":
        import ml_dtypes

        in_np_dt = ml_dtypes.bfloat16
    elif mode == "f16":
        in_np_dt = np.float16
    else:
        in_np_dt = np.float32

    # Host-side layout prep: zero-pad x spatially; build the Winograd weight
    # taps U[t,kw] in lhsT layout [c, (t*KW+kw)*OC + oc]; stripe bias.
    xp = np.zeros((B, C, HP, WP), dtype=in_np_dt)
    xp[:, :, 1:-1, 1:-1] = x
    xp = xp.reshape(N_CORES, B_PER, C, HP * WP)

    g = W.reshape(OC, C, KH, KW)
    g0, g1, g2 = g[:, :, 0, :], g[:, :, 1, :], g[:, :, 2, :]
    u = np.stack(
        [g0, (g0 + g1 + g2) * 0.5, (g0 - g1 + g2) * 0.5, g2], axis=0
    )  # [t, OC, C, KW]
    wt = np.ascontiguousarray(
        u.transpose(2, 0, 3, 1).reshape(C, NTAP * KW * OC)
    ).astype(in_np_dt)
    bias = np.ascontiguousarray(b.reshape(OC_TILES, 128).T).astype(np.float32)

    nc = _get_nc(mode)
    in_maps = [
        {"xp": np.ascontiguousarray(xp[i]), "wt": wt, "bias": bias}
        for i in range(N_CORES)
    ]
    trace = os.environ.get("CONV_TRACE", "") not in ("", "0")
    try:
        res = bass_utils.run_bass_kernel_spmd(
            nc,
            in_maps,
            core_ids=list(range(N_CORES)),
            trace=trace,
        )
    except Exception:
        # transient device wedges (NRT_EXEC_UNIT_UNRECOVERABLE) have been
        # observed once; a fresh dispatch usually recovers
        import time

        time.sleep(2.0)
        res = bass_utils.run_bass_kernel_spmd(
            nc,
            in_maps,
            core_ids=list(range(N_CORES)),
            trace=trace,
        )
    kernel._last_results = res  # for test harness introspection
    out = np.stack([res.results[i]["out"] for i in range(N_CORES)])
    # device emits fp16; repack to the module's fp32 output dtype
    return out.reshape(B, OC, H, W_SP).astype(np.float32)
